# revision 1
# baseline (speedup 1.0000x reference)
"""GAT classifier on 8 trn2 NeuronCores (Bass/Tile).

Sharding: 1D node partition (6250 nodes/core); edges assigned to the core
owning their dst node, sorted by dst into 128-node chunks. Per chunk the
segmented softmax + weighted aggregation is done with PSUM-accumulated
"selection matmuls": S_x[e, j] = (slot_e == j) * x_e built by a dual-op
tensor_scalar against an iota tile, then U[j, :] += S_x^T @ [h_rows | 1].
exp() moves to the dense phase via the factorization
  exp(leaky_relu(as + ad)) = max(u*v, u'*v'),  u = e^as, u' = e^{0.2 as},
                                               v = e^ad, v' = e^{0.2 ad}.
Layer tables (node rows packed [h0|1|h1|1|u,u']) are AllGathered across
cores between layers; pooling partial sums are AllReduced.
"""
import math
import os
import sys
from contextlib import ExitStack
from dataclasses import dataclass

import numpy as np

for _p in ("/opt/trn_rl_repo", "/root/.axon_site/_ro/trn_rl_repo"):
    if os.path.isdir(_p) and _p not in sys.path:
        sys.path.insert(0, _p)

import concourse.bacc as bacc
import concourse.bass as bass
import concourse.mybir as mybir
import concourse.tile as tile
from concourse.tile import add_dep_helper
from concourse.bass_utils import run_bass_kernel_spmd
from concourse.masks import make_identity

P = 128
AF = mybir.ActivationFunctionType
ALU = mybir.AluOpType
F32 = mybir.dt.float32
I16 = mybir.dt.int16


@dataclass
class Cfg:
    N: int = 50000
    E0: int = 800000
    IN: int = 128
    HID: int = 64
    G: int = 512
    CORES: int = 8
    # filled by plan()
    NPC: int = 0
    CH: int = 0
    HALF: int = 0
    SEC_LO: int = 0
    SEC_HI: int = 0
    EC: int = 0
    T: int = 0
    T_LO: int = 0
    G_CH: int = 2

    @property
    def NCH(self):  # padded per-core node count
        return self.CH * P


def plan_cfg(N, E0, G, CORES=8):
    c = Cfg(N=N, E0=E0, G=G, CORES=CORES)
    assert N % CORES == 0
    c.NPC = N // CORES
    c.CH = math.ceil(c.NPC / P)
    c.HALF = ((N // 2) + 127) & ~127  # half-split point for int16 indices
    assert c.HALF < 32768 * 2 and (N - c.HALF) <= 32767 and c.HALF <= 32767
    return c


# ----------------------------------------------------------------- host prep

def prep_edges(cfg, src, dst):
    """Per-core edge arrays. Returns list of dicts + fills cfg.SEC_*/EC/T."""
    owner = dst // cfg.NPC
    per_core = []
    maxlo = maxhi = 0
    for c in range(cfg.CORES):
        m = owner == c
        s = src[m]
        dl = dst[m] - c * cfg.NPC
        chunk = dl >> 7
        half = (s >= cfg.HALF).astype(np.int64)
        order = np.lexsort((s, half, chunk))
        s, dl, chunk, half = s[order], dl[order], chunk[order], half[order]
        key = chunk * 2 + half
        cnt = np.bincount(key, minlength=cfg.CH * 2).reshape(cfg.CH, 2)
        maxlo = max(maxlo, int(cnt[:, 0].max()))
        maxhi = max(maxhi, int(cnt[:, 1].max()))
        per_core.append((s, dl, chunk, half, cnt))
    cfg.SEC_LO = ((maxlo + 127) & ~127) or P
    cfg.SEC_HI = ((maxhi + 127) & ~127) or P
    cfg.EC = cfg.SEC_LO + cfg.SEC_HI
    cfg.T = cfg.EC // P
    cfg.T_LO = cfg.SEC_LO // P

    out = []
    for c in range(cfg.CORES):
        s, dl, chunk, half, cnt = per_core[c]
        gl = np.zeros((cfg.CH, cfg.SEC_LO), np.int16)
        gh = np.zeros((cfg.CH, cfg.SEC_HI), np.int16)
        vi = np.zeros((cfg.CH, cfg.EC), np.int16)
        sl = np.full((cfg.CH, cfg.EC), 300.0, np.float32)
        ofs = np.zeros(cfg.CH * 2 + 1, np.int64)
        np.cumsum(cnt.reshape(-1), out=ofs[1:])
        for k in range(cfg.CH):
            nlo, nhi = int(cnt[k, 0]), int(cnt[k, 1])
            a = ofs[2 * k]
            gl[k, :nlo] = s[a:a + nlo]
            vi[k, :nlo] = dl[a:a + nlo]
            sl[k, :nlo] = (dl[a:a + nlo] & 127).astype(np.float32)
            b = ofs[2 * k + 1]
            gh[k, :nhi] = s[b:b + nhi] - cfg.HALF
            vi[k, cfg.SEC_LO:cfg.SEC_LO + nhi] = dl[b:b + nhi]
            sl[k, cfg.SEC_LO:cfg.SEC_LO + nhi] = (dl[b:b + nhi] & 127).astype(np.float32)

        def wrap16(a):  # idx i -> [i % 16, i // 16], replicated over 8 groups
            w = a.reshape(-1, 16).T.copy()
            return np.tile(w, (8, 1)).astype(np.int16)

        out.append(dict(
            gl=wrap16(gl), gh=wrap16(gh), vi=wrap16(vi),
            slot=sl.reshape(cfg.CH * cfg.T, P).T.copy(),
        ))
    return out


def prep_inputs(cfg, x, edge_index, batch, W1, a_src1, a_dst1, W2, a_src2, a_dst2, fcW):
    N, CORES, NPC, CH = cfg.N, cfg.CORES, cfg.NPC, cfg.CH
    src = np.concatenate([edge_index[0], np.arange(N)]).astype(np.int64)
    dst = np.concatenate([edge_index[1], np.arange(N)]).astype(np.int64)
    edges = prep_edges(cfg, src, dst)

    H = 2
    HID = cfg.HID
    rhs1 = np.zeros((cfg.IN, H * HID + 4), np.float32)
    rhs1[:, :H * HID] = W1
    for h in range(H):
        rhs1[:, H * HID + h] = W1[:, h * HID:(h + 1) * HID] @ a_src1[h]
        rhs1[:, H * HID + 2 + h] = W1[:, h * HID:(h + 1) * HID] @ a_dst1[h]
    rhs2 = np.zeros((H * HID, HID + 2), np.float32)
    rhs2[:, :HID] = W2
    rhs2[:, HID] = W2 @ a_src2[0]
    rhs2[:, HID + 1] = W2 @ a_dst2[0]

    iota128 = np.tile(np.arange(P, dtype=np.float32), (P, 1))
    iota512 = np.tile(np.arange(cfg.G, dtype=np.float32), (P, 1))
    cnt = np.bincount(batch, minlength=cfg.G).astype(np.float32)
    invc = 1.0 / np.maximum(cnt, 1.0)
    invc_b = np.tile(invc, (HID, 1)).astype(np.float32)

    xT = np.zeros((cfg.IN, CORES * cfg.NCH), np.float32)
    xT[:, :0] = 0
    gsl = np.full((CORES, cfg.NCH), 999.0, np.float32)
    for c in range(CORES):
        xT[:, c * cfg.NCH:c * cfg.NCH + NPC] = x[c * NPC:(c + 1) * NPC].T
        gsl[c, :NPC] = batch[c * NPC:(c + 1) * NPC]

    in_maps = []
    for c in range(CORES):
        in_maps.append(dict(
            xT=np.ascontiguousarray(xT[:, c * cfg.NCH:(c + 1) * cfg.NCH]),
            rhs1=rhs1, rhs2=rhs2, fcW=fcW.astype(np.float32),
            iota128=iota128, iota512=iota512, invc=invc_b,
            gslot=gsl[c].reshape(CH, P).T.copy(),
            **edges[c],
        ))
    return in_maps


# -------------------------------------------------------------- bass builder

def build_nc(cfg, stop_after=4):
    N, CH, T, T_LO = cfg.N, cfg.CH, cfg.T, cfg.T_LO
    SEC_LO, SEC_HI, EC, NPC = cfg.SEC_LO, cfg.SEC_HI, cfg.EC, cfg.NPC
    HID, G = cfg.HID, cfg.G
    ROW1 = 192  # [h0(64) 1 h1(64) 1 u u u' u' pad] fp32 -> 768B
    ROW2 = 128  # [h2(64) 1 u u' pad] fp32 -> 512B
    HALF = cfg.HALF
    R = list(range(cfg.CORES))

    nc = bacc.Bacc()
    pi = lambda n, s, d=F32: nc.declare_dram_parameter(n, s, d, isOutput=False)
    xT = pi("xT", [cfg.IN, cfg.NCH])
    rhs1 = pi("rhs1", [cfg.IN, 132])
    rhs2 = pi("rhs2", [2 * HID, HID + 2])
    fcW = pi("fcW", [HID, 2])
    iota128 = pi("iota128", [P, P])
    iota512 = pi("iota512", [P, G])
    invc = pi("invc", [HID, G])
    gslot = pi("gslot", [P, CH])
    gl = pi("gl", [P, CH * SEC_LO // 16], I16)
    gh = pi("gh", [P, CH * SEC_HI // 16], I16)
    vi = pi("vi", [P, CH * EC // 16], I16)
    slot = pi("slot", [P, CH * T])
    out_lg = nc.declare_dram_parameter("out_lg", [G, 2], F32, isOutput=True)

    shard1 = nc.dram_tensor("shard1", [NPC, ROW1], F32)
    table1 = nc.dram_tensor("table1", [N, ROW1], F32, addr_space="Shared")
    vtab1 = nc.dram_tensor("vtab1", [cfg.NCH, 64], F32)
    shard2 = nc.dram_tensor("shard2", [NPC, ROW2], F32)
    table2 = nc.dram_tensor("table2", [N, ROW2], F32, addr_space="Shared")
    vtab2 = nc.dram_tensor("vtab2", [cfg.NCH, 64], F32)
    pool_loc = nc.dram_tensor("pool_loc", [HID, G], F32)
    pool_sh = nc.dram_tensor("pool_sh", [HID, G], F32, addr_space="Shared")

    groups = [tuple(range(a, min(a + cfg.G_CH, CH))) for a in range(0, CH, cfg.G_CH)]

    # SWDGE descriptor-ring pacing: each dma_gather occupies ~num_idxs/16 + 1
    # ring entries until its DMA drains; the ring holds 128 and overrunning it
    # crashes the device. After each gather a 1-element DVE probe-read of its
    # output marks completion; later gathers take a cross-engine dep on the
    # probe so outstanding entries stay under budget.
    gather_fifo = []

    def paced_gather(probe_pool, **kw):
        e = kw["num_idxs"] // 16 + 1
        inst = nc.gpsimd.dma_gather(single_packet=False, **kw)
        gp_t = probe_pool.tile([1, 2], F32, tag="gprobe", name="gprobe")
        rd = nc.vector.tensor_copy(out=gp_t[:], in_=kw["out_ap"][0:1, 0, 0:2])
        tot = sum(x[1] for x in gather_fifo) + e
        while gather_fifo and (tot > 110 or len(gather_fifo) >= 2):
            _, eo, rdo = gather_fifo.pop(0)
            add_dep_helper(inst.ins, rdo.ins, sync=True, reason="swdge ring pacing")
            tot -= eo
        gather_fifo.append((inst, e, rd))
        return inst

    with tile.TileContext(nc) as tc, ExitStack() as ctx:
        cp = ctx.enter_context(tc.tile_pool(name="const", bufs=1))
        dio = ctx.enter_context(tc.tile_pool(name="dio", bufs=3))
        dps = ctx.enter_context(tc.tile_pool(name="dps", bufs=2, space="PSUM"))
        o1p = ctx.enter_context(tc.tile_pool(name="o1p", bufs=1))
        ixp = ctx.enter_context(tc.tile_pool(name="ixp", bufs=2))
        gp = ctx.enter_context(tc.tile_pool(name="gp", bufs=2))
        sxp = ctx.enter_context(tc.tile_pool(name="sxp", bufs=4))
        xp = ctx.enter_context(tc.tile_pool(name="xp", bufs=3))
        ups = ctx.enter_context(tc.tile_pool(name="ups", bufs=2, space="PSUM"))
        pps = ctx.enter_context(tc.tile_pool(name="pps", bufs=1, space="PSUM"))
        fin = ctx.enter_context(tc.tile_pool(name="fin", bufs=3))

        io128 = cp.tile([P, P], F32)
        nc.sync.dma_start(out=io128[:], in_=iota128[:])
        io512 = cp.tile([P, G], F32)
        nc.sync.dma_start(out=io512[:], in_=iota512[:])
        r1sb = cp.tile([cfg.IN, 132], F32)
        nc.sync.dma_start(out=r1sb[:], in_=rhs1[:])
        r2sb = cp.tile([2 * HID, HID + 2], F32)
        nc.sync.dma_start(out=r2sb[:], in_=rhs2[:])
        fcsb = cp.tile([HID, 2], F32)
        nc.sync.dma_start(out=fcsb[:], in_=fcW[:])
        icsb = cp.tile([HID, G], F32)
        nc.sync.dma_start(out=icsb[:], in_=invc[:])
        gssb = cp.tile([P, CH], F32)
        nc.sync.dma_start(out=gssb[:], in_=gslot[:])
        slsb = cp.tile([P, CH * T], F32)
        nc.sync.dma_start(out=slsb[:], in_=slot[:])
        idsb = cp.tile([P, P], F32)
        make_identity(nc, idsb[:])
        out1 = o1p.tile([P, CH * P], F32)

        # ---------------- dense 1: rows of table1 + vtab1 ----------------
        for t in range(CH):
            nv = min(P, NPC - t * P)
            xt = dio.tile([P, P], F32, tag="xt")
            nc.sync.dma_start(out=xt[:], in_=xT[:, t * P:(t + 1) * P])
            ps = dps.tile([P, 132], F32, tag="dtmp")
            nc.tensor.matmul(out=ps[:], lhsT=xt[:], rhs=r1sb[:], start=True, stop=True)
            row = dio.tile([P, ROW1], F32, tag="row1")
            nc.vector.tensor_copy(out=row[:, 0:64], in_=ps[:, 0:64])
            nc.vector.tensor_copy(out=row[:, 65:129], in_=ps[:, 64:128])
            nc.vector.memset(row[:, 64:65], 1.0)
            nc.vector.memset(row[:, 129:130], 1.0)
            nc.scalar.activation(out=row[:, 130:132], in_=ps[:, 128:130], func=AF.Exp, scale=1.0)
            nc.scalar.activation(out=row[:, 132:134], in_=ps[:, 128:130], func=AF.Exp, scale=0.2)
            nc.vector.memset(row[:, 134:192], 0.0)
            vrow = dio.tile([P, 64], F32, tag="vrow")
            nc.scalar.activation(out=vrow[:, 0:2], in_=ps[:, 130:132], func=AF.Exp, scale=1.0)
            nc.scalar.activation(out=vrow[:, 2:4], in_=ps[:, 130:132], func=AF.Exp, scale=0.2)
            nc.vector.memset(vrow[:, 4:64], 0.0)
            nc.sync.dma_start(out=shard1[t * P:t * P + nv, :], in_=row[:nv, :])
            nc.sync.dma_start(out=vtab1[t * P:(t + 1) * P, :], in_=vrow[:])

        tc.strict_bb_all_engine_barrier()
        nc.gpsimd.collective_compute(
            "AllGather", ALU.bypass, replica_groups=[R],
            ins=[shard1[:]], outs=[table1[:]])

        # ---------------- edge phase (shared for both layers) ----------------
        def edge_layer(tabA, tabB, vtab, row_w, nheads, finalize):
            SUB = int(os.environ.get("EDGE_SUB", "4"))
            rw16 = row_w  # elem size in f32 elements
            for grp in groups:
                g0, ng = grp[0], len(grp)
                nlo, nhi, nec = ng * SEC_LO, ng * SEC_HI, ng * EC
                glt = ixp.tile([P, nlo // 16], I16, tag="glt")
                nc.sync.dma_start(out=glt[:], in_=gl[:, g0 * SEC_LO // 16:(g0 * SEC_LO + nlo) // 16])
                ght = ixp.tile([P, nhi // 16], I16, tag="ght")
                nc.sync.dma_start(out=ght[:], in_=gh[:, g0 * SEC_HI // 16:(g0 * SEC_HI + nhi) // 16])
                vit = ixp.tile([P, nec // 16], I16, tag="vit")
                nc.sync.dma_start(out=vit[:], in_=vi[:, g0 * EC // 16:(g0 * EC + nec) // 16])
                hgl = gp.tile([P, nlo // P, rw16], F32, tag="hgl")
                paced_gather(xp, out_ap=hgl[:], in_ap=tabA, idxs_ap=glt[:],
                             num_idxs=nlo, num_idxs_reg=nlo, elem_size=rw16)
                hgh = gp.tile([P, nhi // P, rw16], F32, tag="hgh")
                paced_gather(xp, out_ap=hgh[:], in_ap=tabB, idxs_ap=ght[:],
                             num_idxs=nhi, num_idxs_reg=nhi, elem_size=rw16)
                vg = gp.tile([P, nec // P, 64], F32, tag="vg")
                paced_gather(xp, out_ap=vg[:], in_ap=vtab[:], idxs_ap=vit[:],
                             num_idxs=nec, num_idxs_reg=nec, elem_size=64)
                for ci, c in enumerate(grp):
                    if SUB < 1:
                        continue
                    H2 = 2 * nheads
                    xsb = xp.tile([P, T, 2 * nheads], F32, tag="xsb")
                    m1 = xp.tile([P, T, 2 * nheads], F32, tag="m1")
                    for sec, hg_t, t0, nt in ((0, hgl, 0, T_LO), (1, hgh, T_LO, T - T_LO)):
                        hsl = hg_t[:, ci * nt:(ci + 1) * nt, :]
                        vsl = vg[:, ci * T + t0:ci * T + t0 + nt, :]
                        # u,u' at row cols [64*nheads + nheads + ...]; layout L1: 130..134, L2: 65..67
                        uo = 130 if nheads == 2 else 65
                        nc.vector.tensor_tensor(
                            out=m1[:, t0:t0 + nt, 0:nheads], in0=hsl[:, :, uo:uo + nheads],
                            in1=vsl[:, :, 0:nheads], op=ALU.mult)
                        nc.vector.tensor_tensor(
                            out=xsb[:, t0:t0 + nt, 0:nheads], in0=hsl[:, :, uo + nheads:uo + H2],
                            in1=vsl[:, :, nheads:H2], op=ALU.mult)
                        nc.vector.tensor_tensor(
                            out=xsb[:, t0:t0 + nt, 0:nheads], in0=m1[:, t0:t0 + nt, 0:nheads],
                            in1=xsb[:, t0:t0 + nt, 0:nheads], op=ALU.max)
                    if SUB < 2:
                        continue
                    Us = [ups.tile([P, 65], F32, tag=f"U{h}", name=f"U{h}") for h in range(nheads)]
                    for t in range(T):
                        if t < T_LO:
                            hg_t, tt, nt = hgl, t, T_LO
                        else:
                            hg_t, tt, nt = hgh, t - T_LO, T - T_LO
                        for h in range(nheads):
                            S = sxp.tile([P, P], F32, tag=f"S{h}")
                            nc.vector.tensor_scalar(
                                out=S[:], in0=io128[:],
                                scalar1=slsb[:, c * T + t:c * T + t + 1],
                                scalar2=xsb[:, t, h:h + 1],
                                op0=ALU.is_equal, op1=ALU.mult)
                            if SUB >= 3:
                                nc.tensor.matmul(
                                    out=Us[h][:], lhsT=S[:],
                                    rhs=hg_t[:, ci * nt + tt, h * 65:(h + 1) * 65],
                                    start=(t == 0), stop=(t == T - 1))
                    if SUB >= 4:
                        finalize(c, Us)

        def fin1(c, Us):
            den = fin.tile([P, 2], F32, tag="den1")
            rd = fin.tile([P, 2], F32, tag="rd1")
            for h in range(2):
                nc.vector.tensor_scalar(out=den[:, h:h + 1], in0=Us[h][:, 64:65],
                                        scalar1=1e-20, scalar2=None, op0=ALU.add)
            nc.vector.reciprocal(out=rd[:], in_=den[:])
            for h in range(2):
                nc.vector.tensor_scalar(
                    out=out1[:, c * P + h * 64:c * P + (h + 1) * 64],
                    in0=Us[h][:, 0:64], scalar1=rd[:, h:h + 1], scalar2=0.0,
                    op0=ALU.mult, op1=ALU.max)

        if stop_after >= 2:
            edge_layer(table1[0:HALF, :], table1[HALF:N, :], vtab1, 192, 2, fin1)

        # ---------------- dense 2 ----------------
        for t in range(CH) if stop_after >= 3 else []:
            nv = min(P, NPC - t * P)
            tp = dps.tile([P, P], F32, tag="dtmp")
            nc.tensor.transpose(out=tp[:], in_=out1[:, t * P:(t + 1) * P], identity=idsb[:])
            h1T = dio.tile([P, P], F32, tag="h1T")
            nc.scalar.copy(out=h1T[:], in_=tp[:])
            ps = dps.tile([P, HID + 2], F32, tag="dtmp")
            nc.tensor.matmul(out=ps[:], lhsT=h1T[:], rhs=r2sb[:], start=True, stop=True)
            row = dio.tile([P, ROW2], F32, tag="row2")
            nc.vector.tensor_copy(out=row[:, 0:64], in_=ps[:, 0:64])
            nc.vector.memset(row[:, 64:65], 1.0)
            nc.scalar.activation(out=row[:, 65:66], in_=ps[:, 64:65], func=AF.Exp, scale=1.0)
            nc.scalar.activation(out=row[:, 66:67], in_=ps[:, 64:65], func=AF.Exp, scale=0.2)
            nc.vector.memset(row[:, 67:128], 0.0)
            vrow = dio.tile([P, 64], F32, tag="vrow2")
            nc.scalar.activation(out=vrow[:, 0:1], in_=ps[:, 65:66], func=AF.Exp, scale=1.0)
            nc.scalar.activation(out=vrow[:, 1:2], in_=ps[:, 65:66], func=AF.Exp, scale=0.2)
            nc.vector.memset(vrow[:, 2:64], 0.0)
            nc.sync.dma_start(out=shard2[t * P:t * P + nv, :], in_=row[:nv, :])
            nc.sync.dma_start(out=vtab2[t * P:(t + 1) * P, :], in_=vrow[:])

        if stop_after >= 3:
            tc.strict_bb_all_engine_barrier()
            nc.gpsimd.collective_compute(
                "AllGather", ALU.bypass, replica_groups=[R],
                ins=[shard2[:]], outs=[table2[:]])

        # ---------------- edge layer 2 + pooling ----------------
        plT = pps.tile([HID, G], F32)

        def fin2(c, Us):
            den = fin.tile([P, 1], F32, tag="den2")
            rd = fin.tile([P, 1], F32, tag="rd2")
            nc.vector.tensor_scalar(out=den[:], in0=Us[0][:, 64:65],
                                    scalar1=1e-20, scalar2=None, op0=ALU.add)
            nc.vector.reciprocal(out=rd[:], in_=den[:])
            o2 = fin.tile([P, HID], F32, tag="o2")
            nc.vector.tensor_scalar(out=o2[:], in0=Us[0][:, 0:64],
                                    scalar1=rd[:], scalar2=0.0,
                                    op0=ALU.mult, op1=ALU.max)
            sg = fin.tile([P, G], F32, tag="sg")
            nc.vector.tensor_scalar(out=sg[:], in0=io512[:],
                                    scalar1=gssb[:, c:c + 1], scalar2=None,
                                    op0=ALU.is_equal)
            nc.tensor.matmul(out=plT[:], lhsT=o2[:], rhs=sg[:],
                             start=(c == 0), stop=(c == CH - 1))

        if stop_after >= 4:
            edge_layer(table2[0:HALF, :], table2[HALF:N, :], vtab2, 128, 1, fin2)
        else:
            zz = fin.tile([HID, G], F32, name="zz")
            nc.vector.memset(zz[:], 0.0)
            nc.tensor.matmul(out=plT[:], lhsT=zz[:, 0:P] if HID >= P else zz[:],
                             rhs=zz[:, 0:G], start=True, stop=True) if False else None
            nc.vector.tensor_copy(out=plT[:], in_=zz[:]) if False else None

        plsb = fin.tile([HID, G], F32)
        if stop_after >= 4:
            nc.vector.tensor_copy(out=plsb[:], in_=plT[:])
        else:
            nc.vector.memset(plsb[:], 0.0)
        nc.sync.dma_start(out=pool_loc[:], in_=plsb[:])
        tc.strict_bb_all_engine_barrier()
        nc.gpsimd.collective_compute(
            "AllReduce", ALU.add, replica_groups=[R],
            ins=[pool_loc[:]], outs=[pool_sh[:]])
        plr = fin.tile([HID, G], F32)
        nc.sync.dma_start(out=plr[:], in_=pool_sh[:])
        nc.vector.tensor_tensor(out=plr[:], in0=plr[:], in1=icsb[:], op=ALU.mult)
        for gt in range(max(1, G // P)):
            gw = min(P, G - gt * P)
            lg = dps.tile([P, 2], F32, tag="dtmp")
            nc.tensor.matmul(out=lg[:gw], lhsT=plr[:, gt * P:gt * P + gw], rhs=fcsb[:],
                             start=True, stop=True)
            mx = fin.tile([P, 1], F32, tag="mx")
            nc.vector.tensor_reduce(out=mx[:gw], in_=lg[:gw], op=ALU.max,
                                    axis=mybir.AxisListType.X)
            t1 = fin.tile([P, 2], F32, tag="t1")
            nc.vector.tensor_scalar(out=t1[:gw], in0=lg[:gw], scalar1=mx[:gw],
                                    scalar2=None, op0=ALU.subtract)
            ex = fin.tile([P, 2], F32, tag="ex")
            es = fin.tile([P, 1], F32, tag="es")
            nc.scalar.activation(out=ex[:gw], in_=t1[:gw], func=AF.Exp, accum_out=es[:gw])
            ln = fin.tile([P, 1], F32, tag="ln")
            nc.scalar.activation(out=ln[:gw], in_=es[:gw], func=AF.Ln)
            lsm = fin.tile([P, 2], F32, tag="lsm")
            nc.vector.tensor_scalar(out=lsm[:gw], in0=t1[:gw], scalar1=ln[:gw],
                                    scalar2=None, op0=ALU.subtract)
            nc.sync.dma_start(out=out_lg[gt * P:gt * P + gw, :], in_=lsm[:gw])

    nc.compile()
    return nc


# ------------------------------------------------------------------ entry

LAST_EXEC_NS = None

def kernel(x, edge_index, batch, W1, a_src1, a_dst1, b1, W2, a_src2, a_dst2, b2,
           fcW, fcb):
    x = np.asarray(x, np.float32)
    edge_index = np.asarray(edge_index, np.int64)
    batch = np.asarray(batch, np.int64)
    for b in (b1, b2, fcb):
        assert np.abs(np.asarray(b)).max() == 0.0, "nonzero bias unsupported"
    cfg = plan_cfg(N=x.shape[0], E0=edge_index.shape[1], G=512)
    in_maps = prep_inputs(cfg, x, edge_index, batch,
                          np.asarray(W1, np.float32), np.asarray(a_src1, np.float32),
                          np.asarray(a_dst1, np.float32), np.asarray(W2, np.float32),
                          np.asarray(a_src2, np.float32), np.asarray(a_dst2, np.float32),
                          np.asarray(fcW, np.float32))
    nc = build_nc(cfg)
    trace = os.environ.get("KERNEL_TRACE") == "1"
    res = run_bass_kernel_spmd(nc, in_maps, list(range(cfg.CORES)), trace=trace)
    global LAST_EXEC_NS
    LAST_EXEC_NS = res.exec_time_ns
    if trace:
        print(f"HW exec time: {res.exec_time_ns} ns "
              f"(mean {res.mean_exec_time_ns} ns)")
    return np.asarray(res.results[0]["out_lg"], np.float32)



# revision 13
# speedup vs baseline: 1.7201x; 1.7201x over previous
"""GAT classifier on 8 trn2 NeuronCores (Bass/Tile).

Sharding: 1D node partition (6250 nodes/core, padded to 6272 = 49 chunks of
128 slots); edges assigned to the core owning their dst node, grouped by dst
chunk. Host balances node->chunk assignment on per-half in-degree so the
padded per-(chunk,half) edge-section size is minimal.

Per 128-edge tile the segmented softmax + weighted aggregation is done with
PSUM-accumulated "selection matmuls": S_h[e, j] = (slot_e == j) * w_e with
U[j, :] += S_h^T @ [h_rows | 1]. The edge weight uses the factorization
  exp(leaky_relu(as + ad)) / e^{0.2 ad} = max(u * r_dst, u'),
    u = e^as, u' = e^{0.2 as}, r = e^{0.8 ad},
(the e^{0.2 ad} prefactor cancels in the softmax ratio). u, u' ride along in
the gathered source-node row (fp32 within a bf16 row); r is dst-side local,
broadcast per chunk into [128, 128] tiles via rank-1 matmuls, so no per-edge
v gather is needed. Node tables are bf16 (512B rows L1, 256B rows L2),
AllGathered across cores between layers; pooling partials are AllReduced.
"""
import math
import os
import sys
from contextlib import ExitStack
from dataclasses import dataclass

import numpy as np

for _p in ("/opt/trn_rl_repo", "/root/.axon_site/_ro/trn_rl_repo"):
    if os.path.isdir(_p) and _p not in sys.path:
        sys.path.insert(0, _p)

import concourse.bacc as bacc
import concourse.bass as bass
import concourse.mybir as mybir
import concourse.tile as tile
from concourse.tile import add_dep_helper
from concourse.bass_utils import run_bass_kernel_spmd
from concourse.masks import make_identity

P = 128
AF = mybir.ActivationFunctionType
ALU = mybir.AluOpType
F32 = mybir.dt.float32
BF16 = mybir.dt.bfloat16
I16 = mybir.dt.int16

ROW1 = 256  # bf16: [h0(64) 1 h1(64) 1 pad(2) u0 u1 u0' u1' (4xf32) pad]
ROW2 = 128  # bf16: [h2(64) 1 pad(1) u u' (2xf32) pad]
UO1 = 132   # bf16-elem offset of the fp32 u-block
UO2 = 66


@dataclass
class Cfg:
    N: int = 50000
    E0: int = 800000
    IN: int = 128
    HID: int = 64
    G: int = 512
    CORES: int = 8
    NPC: int = 0
    CH: int = 0
    SEC_LO: int = 0
    SEC_HI: int = 0
    T: int = 0
    T_LO: int = 0
    T_HI: int = 0
    G_CH: int = 2

    @property
    def NCH(self):  # padded per-core node count
        return self.CH * P

    @property
    def HALF(self):  # table rows of cores 0-3 (lo half for int16 indices)
        return (self.CORES // 2) * self.NCH

    @property
    def NTAB(self):
        return self.CORES * self.NCH


def plan_cfg(N, E0, G, CORES=8):
    c = Cfg(N=N, E0=E0, G=G, CORES=CORES)
    assert N % CORES == 0
    c.NPC = N // CORES
    c.CH = math.ceil(c.NPC / P)
    assert c.HALF <= 32767 and (c.NTAB - c.HALF) <= 32768
    return c


# ----------------------------------------------------------------- host prep

def balance_nodes(cfg, src, dst):
    """Assign nodes to (chunk, slot) per core, balancing per-half in-degree
    across chunks. Returns perm_rows: global node id -> global table row."""
    N, NPC, NCH, CH = cfg.N, cfg.NPC, cfg.NCH, cfg.CH
    deg = np.zeros((N, 2), np.int64)
    np.add.at(deg, (dst, ((src // NPC) >= (cfg.CORES // 2)).astype(np.int64)), 1)
    perm_rows = np.empty(N, np.int64)
    for c in range(cfg.CORES):
        dl = deg[c * NPC:(c + 1) * NPC].astype(np.float64)
        order = np.argsort(-(dl[:, 0] + dl[:, 1]), kind="stable")
        loads = np.zeros((CH, 2))
        cnts = np.zeros(CH, np.int64)
        for l in order:
            cand = np.maximum(loads[:, 0] + dl[l, 0], loads[:, 1] + dl[l, 1])
            cand[cnts >= P] = np.inf
            k = int(np.argmin(cand))
            perm_rows[c * NPC + l] = c * NCH + k * P + cnts[k]
            loads[k, 0] += dl[l, 0]
            loads[k, 1] += dl[l, 1]
            cnts[k] += 1
    return perm_rows


def prep_edges(cfg, src, dst, perm_rows):
    """Per-core edge index/slot arrays. Fills cfg.SEC_*/T_*."""
    CH, NPC, NCH, HALF = cfg.CH, cfg.NPC, cfg.NCH, cfg.HALF
    per_core = []
    maxlo = maxhi = 0
    for c in range(cfg.CORES):
        m = (dst // NPC) == c
        srow = perm_rows[src[m]]
        dloc = perm_rows[dst[m]] - c * NCH
        chunk = dloc >> 7
        half = (srow >= HALF).astype(np.int64)
        order = np.lexsort((srow, half, chunk))
        srow, dloc, chunk, half = srow[order], dloc[order], chunk[order], half[order]
        key = chunk * 2 + half
        cnt = np.bincount(key, minlength=CH * 2).reshape(CH, 2)
        maxlo = max(maxlo, int(cnt[:, 0].max()))
        maxhi = max(maxhi, int(cnt[:, 1].max()))
        per_core.append((srow, dloc, cnt))
    cfg.SEC_LO = ((maxlo + 127) & ~127) or P
    cfg.SEC_HI = ((maxhi + 127) & ~127) or P
    cfg.T_LO = cfg.SEC_LO // P
    cfg.T_HI = cfg.SEC_HI // P
    cfg.T = cfg.T_LO + cfg.T_HI
    EC = cfg.SEC_LO + cfg.SEC_HI

    def wrap16(a):  # idx i -> [i % 16, i // 16], replicated over 8 groups
        w = a.reshape(-1, 16).T.copy()
        return np.tile(w, (8, 1)).astype(np.int16)

    out = []
    for c in range(cfg.CORES):
        srow, dloc, cnt = per_core[c]
        gl = np.zeros((CH, cfg.SEC_LO), np.int16)
        gh = np.zeros((CH, cfg.SEC_HI), np.int16)
        sl = np.full((CH, EC), 512.0, np.float32)
        ofs = np.zeros(CH * 2 + 1, np.int64)
        np.cumsum(cnt.reshape(-1), out=ofs[1:])
        for k in range(CH):
            nlo, nhi = int(cnt[k, 0]), int(cnt[k, 1])
            a = ofs[2 * k]
            gl[k, :nlo] = srow[a:a + nlo]
            sl[k, :nlo] = (dloc[a:a + nlo] & 127).astype(np.float32)
            b = ofs[2 * k + 1]
            gh[k, :nhi] = srow[b:b + nhi] - cfg.HALF
            sl[k, cfg.SEC_LO:cfg.SEC_LO + nhi] = (dloc[b:b + nhi] & 127).astype(np.float32)
        out.append(dict(
            gl=wrap16(gl), gh=wrap16(gh),
            slot=sl.reshape(CH * cfg.T, P).T.copy(),
        ))
    return out


def prep_inputs(cfg, x, edge_index, batch, W1, a_src1, a_dst1, W2, a_src2, a_dst2, fcW):
    N, CORES, NPC, NCH, CH = cfg.N, cfg.CORES, cfg.NPC, cfg.NCH, cfg.CH
    # self-loops are NOT in the gathered edge lists — they are applied
    # locally in the finalize step (all their operands are chunk-local)
    src = edge_index[0].astype(np.int64)
    dst = edge_index[1].astype(np.int64)
    perm_rows = balance_nodes(cfg, src, dst)
    edges = prep_edges(cfg, src, dst, perm_rows)

    H = 2
    HID = cfg.HID
    rhs1 = np.zeros((cfg.IN, 132), np.float32)
    rhs1[:, :H * HID] = W1
    for h in range(H):
        rhs1[:, H * HID + h] = W1[:, h * HID:(h + 1) * HID] @ a_src1[h]
        rhs1[:, H * HID + 2 + h] = W1[:, h * HID:(h + 1) * HID] @ a_dst1[h]
    rhs2 = np.zeros((H * HID, HID + 2), np.float32)
    rhs2[:, :HID] = W2
    rhs2[:, HID] = W2 @ a_src2[0]
    rhs2[:, HID + 1] = W2 @ a_dst2[0]

    iota128 = np.tile(np.arange(P, dtype=np.float32), (P, 1))
    iota512 = np.tile(np.arange(cfg.G, dtype=np.float32), (P, 1))
    cnt = np.bincount(batch, minlength=cfg.G).astype(np.float32)
    invc_b = np.tile(1.0 / np.maximum(cnt, 1.0), (HID, 1)).astype(np.float32)

    xT = np.zeros((cfg.IN, CORES * NCH), np.float32)
    gsl = np.full((CORES, NCH), 999.0, np.float32)
    for c in range(CORES):
        nodes = np.arange(c * NPC, (c + 1) * NPC)
        rows = perm_rows[nodes] - c * NCH
        xT[:, c * NCH + rows] = x[nodes].T
        gsl[c, rows] = batch[nodes]

    in_maps = []
    for c in range(CORES):
        in_maps.append(dict(
            xT=np.ascontiguousarray(xT[:, c * NCH:(c + 1) * NCH]),
            rhs1=rhs1, rhs2=rhs2, fcW=fcW.astype(np.float32),
            iota128=iota128, iota512=iota512, invc=invc_b,
            gslot=gsl[c].reshape(CH, P).T.copy(),
            **edges[c],
        ))
    return in_maps


# -------------------------------------------------------------- bass builder

def build_nc(cfg):
    CH, T, T_LO, T_HI = cfg.CH, cfg.T, cfg.T_LO, cfg.T_HI
    SEC_LO, SEC_HI = cfg.SEC_LO, cfg.SEC_HI
    HID, G, NCH, HALF, NTAB = cfg.HID, cfg.G, cfg.NCH, cfg.HALF, cfg.NTAB
    R = list(range(cfg.CORES))

    nc = bacc.Bacc()
    pi = lambda n, s, d=F32: nc.declare_dram_parameter(n, s, d, isOutput=False)
    xT = pi("xT", [cfg.IN, NCH])
    rhs1 = pi("rhs1", [cfg.IN, 132])
    rhs2 = pi("rhs2", [2 * HID, HID + 2])
    fcW = pi("fcW", [HID, 2])
    iota128 = pi("iota128", [P, P])
    iota512 = pi("iota512", [P, G])
    invc = pi("invc", [HID, G])
    gslot = pi("gslot", [P, CH])
    gl = pi("gl", [P, CH * SEC_LO // 16], I16)
    gh = pi("gh", [P, CH * SEC_HI // 16], I16)
    slot = pi("slot", [P, CH * T])
    out_lg = nc.declare_dram_parameter("out_lg", [G, 2], F32, isOutput=True)

    shard1 = nc.dram_tensor("shard1", [NCH, ROW1], BF16)
    table1 = nc.dram_tensor("table1", [NTAB, ROW1], BF16, addr_space="Shared")
    shard2 = nc.dram_tensor("shard2", [NCH, ROW2], BF16)
    table2 = nc.dram_tensor("table2", [NTAB, ROW2], BF16, addr_space="Shared")
    pool_loc = nc.dram_tensor("pool_loc", [HID, G], F32)
    pool_sh = nc.dram_tensor("pool_sh", [HID, G], F32, addr_space="Shared")

    groups = [tuple(range(a, min(a + cfg.G_CH, CH))) for a in range(0, CH, cfg.G_CH)]

    # SWDGE descriptor-ring pacing (see baseline): probe-read marks gather
    # completion; later gathers dep on the probe to bound outstanding entries.
    gather_fifo = []

    def paced_gather(probe_pool, **kw):
        e = kw["num_idxs"] // 16 + 1
        inst = nc.gpsimd.dma_gather(single_packet=False, **kw)
        gp_t = probe_pool.tile([1, 2], F32, tag="gprobe", name="gprobe")
        rd = nc.vector.tensor_copy(out=gp_t[:], in_=kw["out_ap"][0:1, 0, 0:2])
        tot = sum(x[1] for x in gather_fifo) + e
        while gather_fifo and (tot > 110 or len(gather_fifo) >= 2):
            _, eo, rdo = gather_fifo.pop(0)
            add_dep_helper(inst.ins, rdo.ins, sync=True, reason="swdge ring pacing")
            tot -= eo
        gather_fifo.append((inst, e, rd))
        return inst

    with tile.TileContext(nc) as tc, ExitStack() as ctx:
        cp = ctx.enter_context(tc.tile_pool(name="const", bufs=1))
        dio = ctx.enter_context(tc.tile_pool(name="dio", bufs=3))
        dps = ctx.enter_context(tc.tile_pool(name="dps", bufs=2, space="PSUM"))
        o1p = ctx.enter_context(tc.tile_pool(name="o1p", bufs=1))
        ixp = ctx.enter_context(tc.tile_pool(name="ixp", bufs=2))
        gp = ctx.enter_context(tc.tile_pool(name="gp", bufs=2))
        sxp = ctx.enter_context(tc.tile_pool(name="sxp", bufs=6))
        xp = ctx.enter_context(tc.tile_pool(name="xp", bufs=3))
        ups = ctx.enter_context(tc.tile_pool(name="ups", bufs=2, space="PSUM"))
        pps = ctx.enter_context(tc.tile_pool(name="pps", bufs=1, space="PSUM"))
        fin = ctx.enter_context(tc.tile_pool(name="fin", bufs=3))

        io128 = cp.tile([P, P], F32)
        nc.sync.dma_start(out=io128[:], in_=iota128[:])
        io512 = cp.tile([P, G], F32)
        nc.sync.dma_start(out=io512[:], in_=iota512[:])
        r1sb = cp.tile([cfg.IN, 132], F32)
        nc.sync.dma_start(out=r1sb[:], in_=rhs1[:])
        r2sb = cp.tile([2 * HID, HID + 2], F32)
        nc.sync.dma_start(out=r2sb[:], in_=rhs2[:])
        fcsb = cp.tile([HID, 2], F32)
        nc.sync.dma_start(out=fcsb[:], in_=fcW[:])
        icsb = cp.tile([HID, G], F32)
        nc.sync.dma_start(out=icsb[:], in_=invc[:])
        gssb = cp.tile([P, CH], F32)
        nc.sync.dma_start(out=gssb[:], in_=gslot[:])
        slsb = cp.tile([P, CH * T], F32)
        nc.sync.dma_start(out=slsb[:], in_=slot[:])
        idsb = cp.tile([P, P], F32)
        make_identity(nc, idsb[:])
        onesW = cp.tile([P, P], F32)
        nc.vector.memset(onesW[:], 1.0)
        out1 = o1p.tile([P, CH * P], F32)
        VB1 = cp.tile([P, CH * 2 * P], BF16)
        VB2 = cp.tile([P, CH * P], BF16)
        rcol1 = cp.tile([P, 2 * CH], F32)
        rcol2 = cp.tile([P, CH], F32)
        rows1 = cp.tile([P, CH * ROW1], BF16)  # local node rows (self-loop term)
        rows2 = cp.tile([P, CH * ROW2], BF16)
        nc.vector.memset(rows1[:], 0.0)
        nc.vector.memset(rows2[:], 0.0)

        # ---------------- dense 1: rows of table1 + r columns ----------------
        for t in range(CH):
            xt = dio.tile([P, P], F32, tag="xt")
            nc.sync.dma_start(out=xt[:], in_=xT[:, t * P:(t + 1) * P])
            ps = dps.tile([P, 132], F32, tag="dtmp")
            nc.tensor.matmul(out=ps[:], lhsT=xt[:], rhs=r1sb[:], start=True, stop=True)
            row = rows1[:, t * ROW1:(t + 1) * ROW1]
            nc.vector.tensor_copy(out=row[:, 0:64], in_=ps[:, 0:64])
            nc.vector.memset(row[:, 64:65], 1.0)
            nc.vector.tensor_copy(out=row[:, 65:129], in_=ps[:, 64:128])
            nc.vector.memset(row[:, 129:130], 1.0)
            um = row[:, UO1:UO1 + 8].bitcast(F32)
            nc.scalar.activation(out=um[:, 0:2], in_=ps[:, 128:130], func=AF.Exp, scale=1.0)
            nc.scalar.activation(out=um[:, 2:4], in_=ps[:, 128:130], func=AF.Exp, scale=0.2)
            nc.scalar.activation(out=rcol1[:, 2 * t:2 * t + 2], in_=ps[:, 130:132],
                                 func=AF.Exp, scale=0.8)
            nc.sync.dma_start(out=shard1[t * P:(t + 1) * P, :], in_=row[:])

        # VB: per (chunk, head) tile with r along the free dim — replicate the
        # r column along free (tensor_scalar vs ones), PE-transpose, copy bf16.
        def build_vb(rcol_t, ncols, vb_t):
            for i in range(ncols):
                rc = fin.tile([P, P], F32, tag="rc")
                nc.vector.tensor_scalar(out=rc[:], in0=onesW[:],
                                        scalar1=rcol_t[:, i:i + 1], scalar2=None,
                                        op0=ALU.mult)
                vps = dps.tile([P, P], F32, tag="dtmp")
                nc.tensor.transpose(out=vps[:], in_=rc[:], identity=idsb[:])
                nc.scalar.copy(out=vb_t[:, i * P:(i + 1) * P], in_=vps[:])

        build_vb(rcol1, 2 * CH, VB1)

        tc.strict_bb_all_engine_barrier()
        nc.gpsimd.collective_compute(
            "AllGather", ALU.bypass, replica_groups=[R],
            ins=[shard1[:]], outs=[table1[:]])

        # ---------------- edge phase (shared for both layers) ----------------
        def edge_layer(tabA, tabB, vb_t, row_w, uo, nheads, finalize):
            for grp in groups:
                g0, ng = grp[0], len(grp)
                nlo, nhi = ng * SEC_LO, ng * SEC_HI
                glt = ixp.tile([P, nlo // 16], I16, tag="glt")
                nc.sync.dma_start(out=glt[:], in_=gl[:, g0 * SEC_LO // 16:(g0 * SEC_LO + nlo) // 16])
                ght = ixp.tile([P, nhi // 16], I16, tag="ght")
                nc.sync.dma_start(out=ght[:], in_=gh[:, g0 * SEC_HI // 16:(g0 * SEC_HI + nhi) // 16])
                hgl = gp.tile([P, nlo // P, row_w], BF16, tag="hgl")
                paced_gather(xp, out_ap=hgl[:], in_ap=tabA, idxs_ap=glt[:],
                             num_idxs=nlo, num_idxs_reg=nlo, elem_size=row_w)
                hgh = gp.tile([P, nhi // P, row_w], BF16, tag="hgh")
                paced_gather(xp, out_ap=hgh[:], in_ap=tabB, idxs_ap=ght[:],
                             num_idxs=nhi, num_idxs_reg=nhi, elem_size=row_w)
                for ci, c in enumerate(grp):
                    Us = [ups.tile([P, 65], F32, tag=f"U{h}", name=f"U{h}")
                          for h in range(nheads)]
                    for t in range(T):
                        if t < T_LO:
                            hg_t, tt, nt = hgl, t, T_LO
                        else:
                            hg_t, tt, nt = hgh, t - T_LO, T_HI
                        base = ci * nt + tt
                        S0 = sxp.tile([P, P], BF16, tag="S0")
                        nc.vector.tensor_scalar(
                            out=S0[:], in0=io128[:],
                            scalar1=slsb[:, c * T + t:c * T + t + 1],
                            scalar2=None, op0=ALU.is_equal)
                        uf = hg_t[:, base, uo:uo + 4 * nheads].bitcast(F32)
                        for h in range(nheads):
                            Th = sxp.tile([P, P], BF16, tag=f"T{h}")
                            nc.vector.tensor_scalar(
                                out=Th[:],
                                in0=vb_t[:, (c * nheads + h) * P:(c * nheads + h + 1) * P],
                                scalar1=uf[:, h:h + 1],
                                scalar2=uf[:, nheads + h:nheads + h + 1],
                                op0=ALU.mult, op1=ALU.max)
                            Sh = sxp.tile([P, P], BF16, tag=f"S{h}")
                            nc.vector.tensor_tensor(out=Sh[:], in0=S0[:], in1=Th[:],
                                                    op=ALU.mult)
                            nc.tensor.matmul(
                                out=Us[h][:], lhsT=Sh[:],
                                rhs=hg_t[:, base, h * 65:(h + 1) * 65],
                                start=(t == 0), stop=(t == T - 1))
                    finalize(c, Us)

        # self-loop term: w = max(u*r, u'), numerator += w*h_local, den += w
        def self_w(c, h, rows_t, rcol_t, row_w, uo, nheads):
            uf = rows_t[:, c * row_w + uo:c * row_w + uo + 4 * nheads].bitcast(F32)
            t1 = fin.tile([P, 1], F32, tag="t1w")
            nc.vector.tensor_tensor(out=t1[:], in0=uf[:, h:h + 1],
                                    in1=rcol_t[:, nheads * c + h:nheads * c + h + 1],
                                    op=ALU.mult)
            ws = fin.tile([P, 1], F32, tag="ws")
            nc.vector.tensor_tensor(out=ws[:], in0=t1[:], in1=uf[:, nheads + h:nheads + h + 1],
                                    op=ALU.max)
            sh = fin.tile([P, 64], F32, tag="shw")
            nc.vector.tensor_scalar(out=sh[:], in0=rows_t[:, c * row_w + h * 65:c * row_w + h * 65 + 64],
                                    scalar1=ws[:], scalar2=None, op0=ALU.mult)
            return ws, sh

        def fin1(c, Us):
            for h in range(2):
                ws, sh = self_w(c, h, rows1, rcol1, ROW1, UO1, 2)
                un = fin.tile([P, 64], F32, tag="un1")
                nc.vector.tensor_tensor(out=un[:], in0=Us[h][:, 0:64], in1=sh[:],
                                        op=ALU.add)
                den = fin.tile([P, 1], F32, tag="den1")
                nc.vector.tensor_tensor(out=den[:], in0=Us[h][:, 64:65], in1=ws[:],
                                        op=ALU.add)
                rd = fin.tile([P, 1], F32, tag="rd1")
                nc.vector.reciprocal(out=rd[:], in_=den[:])
                nc.vector.tensor_scalar(
                    out=out1[:, c * P + h * 64:c * P + (h + 1) * 64],
                    in0=un[:], scalar1=rd[:], scalar2=0.0,
                    op0=ALU.mult, op1=ALU.max)

        edge_layer(table1[0:HALF, :], table1[HALF:NTAB, :], VB1, ROW1, UO1, 2, fin1)

        # ---------------- dense 2 ----------------
        for t in range(CH):
            tp = dps.tile([P, P], F32, tag="dtmp")
            nc.tensor.transpose(out=tp[:], in_=out1[:, t * P:(t + 1) * P], identity=idsb[:])
            h1T = dio.tile([P, P], F32, tag="h1T")
            nc.scalar.copy(out=h1T[:], in_=tp[:])
            ps = dps.tile([P, HID + 2], F32, tag="dtmp")
            nc.tensor.matmul(out=ps[:], lhsT=h1T[:], rhs=r2sb[:], start=True, stop=True)
            row = rows2[:, t * ROW2:(t + 1) * ROW2]
            nc.vector.tensor_copy(out=row[:, 0:64], in_=ps[:, 0:64])
            nc.vector.memset(row[:, 64:65], 1.0)
            um = row[:, UO2:UO2 + 4].bitcast(F32)
            nc.scalar.activation(out=um[:, 0:1], in_=ps[:, 64:65], func=AF.Exp, scale=1.0)
            nc.scalar.activation(out=um[:, 1:2], in_=ps[:, 64:65], func=AF.Exp, scale=0.2)
            nc.scalar.activation(out=rcol2[:, t:t + 1], in_=ps[:, 65:66],
                                 func=AF.Exp, scale=0.8)
            nc.sync.dma_start(out=shard2[t * P:(t + 1) * P, :], in_=row[:])

        build_vb(rcol2, CH, VB2)

        tc.strict_bb_all_engine_barrier()
        nc.gpsimd.collective_compute(
            "AllGather", ALU.bypass, replica_groups=[R],
            ins=[shard2[:]], outs=[table2[:]])

        # ---------------- edge layer 2 + pooling ----------------
        plT = pps.tile([HID, G], F32)

        def fin2(c, Us):
            ws, sh = self_w(c, 0, rows2, rcol2, ROW2, UO2, 1)
            un = fin.tile([P, 64], F32, tag="un2")
            nc.vector.tensor_tensor(out=un[:], in0=Us[0][:, 0:64], in1=sh[:],
                                    op=ALU.add)
            den = fin.tile([P, 1], F32, tag="den2")
            nc.vector.tensor_tensor(out=den[:], in0=Us[0][:, 64:65], in1=ws[:],
                                    op=ALU.add)
            rd = fin.tile([P, 1], F32, tag="rd2")
            nc.vector.reciprocal(out=rd[:], in_=den[:])
            o2 = fin.tile([P, HID], F32, tag="o2")
            nc.vector.tensor_scalar(out=o2[:], in0=un[:],
                                    scalar1=rd[:], scalar2=0.0,
                                    op0=ALU.mult, op1=ALU.max)
            sg = fin.tile([P, G], F32, tag="sg")
            nc.vector.tensor_scalar(out=sg[:], in0=io512[:],
                                    scalar1=gssb[:, c:c + 1], scalar2=None,
                                    op0=ALU.is_equal)
            nc.tensor.matmul(out=plT[:], lhsT=o2[:], rhs=sg[:],
                             start=(c == 0), stop=(c == CH - 1))

        edge_layer(table2[0:HALF, :], table2[HALF:NTAB, :], VB2, ROW2, UO2, 1, fin2)

        plsb = fin.tile([HID, G], F32)
        nc.vector.tensor_copy(out=plsb[:], in_=plT[:])
        nc.sync.dma_start(out=pool_loc[:], in_=plsb[:])
        tc.strict_bb_all_engine_barrier()
        nc.gpsimd.collective_compute(
            "AllReduce", ALU.add, replica_groups=[R],
            ins=[pool_loc[:]], outs=[pool_sh[:]])
        plr = fin.tile([HID, G], F32)
        nc.sync.dma_start(out=plr[:], in_=pool_sh[:])
        nc.vector.tensor_tensor(out=plr[:], in0=plr[:], in1=icsb[:], op=ALU.mult)
        for gt in range(max(1, G // P)):
            gw = min(P, G - gt * P)
            lg = dps.tile([P, 2], F32, tag="dtmp")
            nc.tensor.matmul(out=lg[:gw], lhsT=plr[:, gt * P:gt * P + gw], rhs=fcsb[:],
                             start=True, stop=True)
            mx = fin.tile([P, 1], F32, tag="mx")
            nc.vector.tensor_reduce(out=mx[:gw], in_=lg[:gw], op=ALU.max,
                                    axis=mybir.AxisListType.X)
            t1 = fin.tile([P, 2], F32, tag="t1")
            nc.vector.tensor_scalar(out=t1[:gw], in0=lg[:gw], scalar1=mx[:gw],
                                    scalar2=None, op0=ALU.subtract)
            ex = fin.tile([P, 2], F32, tag="ex")
            es = fin.tile([P, 1], F32, tag="es")
            nc.scalar.activation(out=ex[:gw], in_=t1[:gw], func=AF.Exp, accum_out=es[:gw])
            ln = fin.tile([P, 1], F32, tag="ln")
            nc.scalar.activation(out=ln[:gw], in_=es[:gw], func=AF.Ln)
            lsm = fin.tile([P, 2], F32, tag="lsm")
            nc.vector.tensor_scalar(out=lsm[:gw], in0=t1[:gw], scalar1=ln[:gw],
                                    scalar2=None, op0=ALU.subtract)
            nc.sync.dma_start(out=out_lg[gt * P:gt * P + gw, :], in_=lsm[:gw])

    nc.compile()
    return nc


# ------------------------------------------------------------------ entry

LAST_EXEC_NS = None

def kernel(x, edge_index, batch, W1, a_src1, a_dst1, b1, W2, a_src2, a_dst2, b2,
           fcW, fcb):
    x = np.asarray(x, np.float32)
    edge_index = np.asarray(edge_index, np.int64)
    batch = np.asarray(batch, np.int64)
    for b in (b1, b2, fcb):
        assert np.abs(np.asarray(b)).max() == 0.0, "nonzero bias unsupported"
    cfg = plan_cfg(N=x.shape[0], E0=edge_index.shape[1], G=512)
    in_maps = prep_inputs(cfg, x, edge_index, batch,
                          np.asarray(W1, np.float32), np.asarray(a_src1, np.float32),
                          np.asarray(a_dst1, np.float32), np.asarray(W2, np.float32),
                          np.asarray(a_src2, np.float32), np.asarray(a_dst2, np.float32),
                          np.asarray(fcW, np.float32))
    nc = build_nc(cfg)
    trace = os.environ.get("KERNEL_TRACE") == "1"
    res = run_bass_kernel_spmd(nc, in_maps, list(range(cfg.CORES)), trace=trace)
    global LAST_EXEC_NS
    LAST_EXEC_NS = res.exec_time_ns
    if trace:
        print(f"HW exec time: {res.exec_time_ns} ns "
              f"(mean {res.mean_exec_time_ns} ns)")
    return np.asarray(res.results[0]["out_lg"], np.float32)


# revision 14
# speedup vs baseline: 1.7867x; 1.0387x over previous
"""GAT classifier on 8 trn2 NeuronCores (Bass/Tile).

Sharding: 1D node partition (6250 nodes/core, padded to 6272 = 49 chunks of
128 slots); edges assigned to the core owning their dst node, grouped by dst
chunk. Host balances node->chunk assignment on per-half in-degree so the
padded per-(chunk,half) edge-section size is minimal. Self-loops are applied
locally in the finalize step (operands are chunk-local), not gathered.

Per 128-edge tile the segmented softmax + weighted aggregation is one
PSUM-accumulated "selection matmul": U[j, :] += Sh^T @ [u'h_rows | u'],
Sh[e, j] = S0[e, j] * wt[e, j], where S0[e, j] = (slot_e == j) is a STATIC
0/1 mask streamed from the host, and the weight uses the factorization
  exp(leaky_relu(as + ad)) / e^{0.2(as+ad)} = e^{0.8 relu(as + ad)} = wt
(the e^{0.2 as} factor is premultiplied into the source row as u' = e^{0.2 as},
and e^{0.2 ad} cancels in the softmax ratio). wt is built per tile by the
SCALAR engine from a per-chunk broadcast tile AD[e, j] = ad_j and a
per-partition bias as_e carried in the gathered row (fp32 inside the bf16
row), then masked with one DVE tensor_tensor. Node tables are bf16 (512B
rows L1, 256B rows L2), AllGathered between layers; pooling partials are
AllReduced.
"""
import math
import os
import sys
from contextlib import ExitStack
from dataclasses import dataclass

import numpy as np
import ml_dtypes

for _p in ("/opt/trn_rl_repo", "/root/.axon_site/_ro/trn_rl_repo"):
    if os.path.isdir(_p) and _p not in sys.path:
        sys.path.insert(0, _p)

import concourse.bacc as bacc
import concourse.bass as bass
import concourse.mybir as mybir
import concourse.tile as tile
from concourse.tile import add_dep_helper
from concourse.bass_utils import run_bass_kernel_spmd
from concourse.masks import make_identity

P = 128
AF = mybir.ActivationFunctionType
ALU = mybir.AluOpType
F32 = mybir.dt.float32
BF16 = mybir.dt.bfloat16
I16 = mybir.dt.int16
BF = ml_dtypes.bfloat16

ROW1 = 256  # bf16: [u'0*h0(64) u'0 u'1*h1(64) u'1 pad(2) f32:{as0 as1 as08_0 as08_1 u'0 u'1}]
ROW2 = 128  # bf16: [u'*h2(64) u' pad(1) f32:{as as08 u'}]
UO1 = 132   # bf16-elem offset of the fp32 block
UO2 = 66

# weight-build variant per (layer, head): "relu2" = 2 scalar acts + 1 DVE TT;
# "exp1" = 1 scalar act + 1 DVE TS + 1 DVE TT. Chosen to balance Scalar vs DVE.
VARIANT = {(1, 0): "relu2", (1, 1): "exp1", (2, 0): "exp1"}


@dataclass
class Cfg:
    N: int = 50000
    E0: int = 800000
    IN: int = 128
    HID: int = 64
    G: int = 512
    CORES: int = 8
    NPC: int = 0
    CH: int = 0
    SEC_LO: int = 0
    SEC_HI: int = 0
    T: int = 0
    T_LO: int = 0
    T_HI: int = 0
    G_CH: int = 2

    @property
    def NCH(self):  # padded per-core node count
        return self.CH * P

    @property
    def HALF(self):  # table rows of cores 0-3 (lo half for int16 indices)
        return (self.CORES // 2) * self.NCH

    @property
    def NTAB(self):
        return self.CORES * self.NCH


def plan_cfg(N, E0, G, CORES=8):
    c = Cfg(N=N, E0=E0, G=G, CORES=CORES)
    assert N % CORES == 0
    c.NPC = N // CORES
    c.CH = math.ceil(c.NPC / P)
    assert c.HALF <= 32767 and (c.NTAB - c.HALF) <= 32768
    return c


# ----------------------------------------------------------------- host prep

def balance_nodes(cfg, src, dst):
    """Assign nodes to (chunk, slot) per core, balancing per-half in-degree
    across chunks. Returns perm_rows: global node id -> global table row."""
    N, NPC, NCH, CH = cfg.N, cfg.NPC, cfg.NCH, cfg.CH
    deg = np.zeros((N, 2), np.int64)
    np.add.at(deg, (dst, ((src // NPC) >= (cfg.CORES // 2)).astype(np.int64)), 1)
    perm_rows = np.empty(N, np.int64)
    for c in range(cfg.CORES):
        dl = deg[c * NPC:(c + 1) * NPC].astype(np.float64)
        order = np.argsort(-(dl[:, 0] + dl[:, 1]), kind="stable")
        loads = np.zeros((CH, 2))
        cnts = np.zeros(CH, np.int64)
        for l in order:
            cand = np.maximum(loads[:, 0] + dl[l, 0], loads[:, 1] + dl[l, 1])
            cand[cnts >= P] = np.inf
            k = int(np.argmin(cand))
            perm_rows[c * NPC + l] = c * NCH + k * P + cnts[k]
            loads[k, 0] += dl[l, 0]
            loads[k, 1] += dl[l, 1]
            cnts[k] += 1
    return perm_rows


def prep_edges(cfg, src, dst, perm_rows):
    """Per-core edge index arrays + static S0 mask tiles. Fills cfg.SEC_*/T_*."""
    CH, NPC, NCH, HALF = cfg.CH, cfg.NPC, cfg.NCH, cfg.HALF
    per_core = []
    maxlo = maxhi = 0
    for c in range(cfg.CORES):
        m = (dst // NPC) == c
        srow = perm_rows[src[m]]
        dloc = perm_rows[dst[m]] - c * NCH
        chunk = dloc >> 7
        half = (srow >= HALF).astype(np.int64)
        order = np.lexsort((srow, half, chunk))
        srow, dloc, chunk, half = srow[order], dloc[order], chunk[order], half[order]
        key = chunk * 2 + half
        cnt = np.bincount(key, minlength=CH * 2).reshape(CH, 2)
        maxlo = max(maxlo, int(cnt[:, 0].max()))
        maxhi = max(maxhi, int(cnt[:, 1].max()))
        per_core.append((srow, dloc, cnt))
    cfg.SEC_LO = ((maxlo + 127) & ~127) or P
    cfg.SEC_HI = ((maxhi + 127) & ~127) or P
    cfg.T_LO = cfg.SEC_LO // P
    cfg.T_HI = cfg.SEC_HI // P
    cfg.T = cfg.T_LO + cfg.T_HI
    EC = cfg.SEC_LO + cfg.SEC_HI

    def wrap16(a):  # idx i -> [i % 16, i // 16], replicated over 8 groups
        w = a.reshape(-1, 16).T.copy()
        return np.tile(w, (8, 1)).astype(np.int16)

    jj = np.arange(P, dtype=np.int64)
    out = []
    for c in range(cfg.CORES):
        srow, dloc, cnt = per_core[c]
        gl = np.zeros((CH, cfg.SEC_LO), np.int16)
        gh = np.zeros((CH, cfg.SEC_HI), np.int16)
        sl = np.full((CH, EC), 512, np.int64)
        ofs = np.zeros(CH * 2 + 1, np.int64)
        np.cumsum(cnt.reshape(-1), out=ofs[1:])
        for k in range(CH):
            nlo, nhi = int(cnt[k, 0]), int(cnt[k, 1])
            a = ofs[2 * k]
            gl[k, :nlo] = srow[a:a + nlo]
            sl[k, :nlo] = dloc[a:a + nlo] & 127
            b = ofs[2 * k + 1]
            gh[k, :nhi] = srow[b:b + nhi] - cfg.HALF
            sl[k, cfg.SEC_LO:cfg.SEC_LO + nhi] = dloc[b:b + nhi] & 127
        # S0 mask tiles: [e, tile*128 + j] = (slot == j), tile-major
        s0 = (sl.reshape(CH * cfg.T, P)[:, :, None] == jj[None, None, :])
        s0 = np.ascontiguousarray(
            s0.transpose(1, 0, 2).reshape(P, CH * cfg.T * P)).astype(BF)
        out.append(dict(gl=wrap16(gl), gh=wrap16(gh), s0=s0))
    return out


def prep_inputs(cfg, x, edge_index, batch, W1, a_src1, a_dst1, W2, a_src2, a_dst2, fcW):
    N, CORES, NPC, NCH, CH = cfg.N, cfg.CORES, cfg.NPC, cfg.NCH, cfg.CH
    # self-loops are NOT in the gathered edge lists — they are applied
    # locally in the finalize step (all their operands are chunk-local)
    src = edge_index[0].astype(np.int64)
    dst = edge_index[1].astype(np.int64)
    perm_rows = balance_nodes(cfg, src, dst)
    edges = prep_edges(cfg, src, dst, perm_rows)

    H = 2
    HID = cfg.HID
    rhs1 = np.zeros((cfg.IN, 132), np.float32)
    rhs1[:, :H * HID] = W1
    for h in range(H):
        rhs1[:, H * HID + h] = W1[:, h * HID:(h + 1) * HID] @ a_src1[h]
        rhs1[:, H * HID + 2 + h] = W1[:, h * HID:(h + 1) * HID] @ a_dst1[h]
    rhs2 = np.zeros((H * HID, HID + 2), np.float32)
    rhs2[:, :HID] = W2
    rhs2[:, HID] = W2 @ a_src2[0]
    rhs2[:, HID + 1] = W2 @ a_dst2[0]

    iota512 = np.tile(np.arange(cfg.G, dtype=np.float32), (P, 1))
    cnt = np.bincount(batch, minlength=cfg.G).astype(np.float32)
    invc_b = np.tile(1.0 / np.maximum(cnt, 1.0), (HID, 1)).astype(np.float32)

    xT = np.zeros((cfg.IN, CORES * NCH), np.float32)
    gsl = np.full((CORES, NCH), 999.0, np.float32)
    for c in range(CORES):
        nodes = np.arange(c * NPC, (c + 1) * NPC)
        rows = perm_rows[nodes] - c * NCH
        xT[:, c * NCH + rows] = x[nodes].T
        gsl[c, rows] = batch[nodes]

    in_maps = []
    for c in range(CORES):
        in_maps.append(dict(
            xT=np.ascontiguousarray(xT[:, c * NCH:(c + 1) * NCH]),
            rhs1=rhs1, rhs2=rhs2, fcW=fcW.astype(np.float32),
            iota512=iota512, invc=invc_b,
            gslot=gsl[c].reshape(CH, P).T.copy(),
            **edges[c],
        ))
    return in_maps


# -------------------------------------------------------------- bass builder

def build_nc(cfg):
    CH, T, T_LO, T_HI = cfg.CH, cfg.T, cfg.T_LO, cfg.T_HI
    SEC_LO, SEC_HI = cfg.SEC_LO, cfg.SEC_HI
    HID, G, NCH, HALF, NTAB = cfg.HID, cfg.G, cfg.NCH, cfg.HALF, cfg.NTAB
    R = list(range(cfg.CORES))

    nc = bacc.Bacc()
    pi = lambda n, s, d=F32: nc.declare_dram_parameter(n, s, d, isOutput=False)
    xT = pi("xT", [cfg.IN, NCH])
    rhs1 = pi("rhs1", [cfg.IN, 132])
    rhs2 = pi("rhs2", [2 * HID, HID + 2])
    fcW = pi("fcW", [HID, 2])
    iota512 = pi("iota512", [P, G])
    invc = pi("invc", [HID, G])
    gslot = pi("gslot", [P, CH])
    gl = pi("gl", [P, CH * SEC_LO // 16], I16)
    gh = pi("gh", [P, CH * SEC_HI // 16], I16)
    s0p = pi("s0", [P, CH * T * P], BF16)
    out_lg = nc.declare_dram_parameter("out_lg", [G, 2], F32, isOutput=True)

    shard1 = nc.dram_tensor("shard1", [NCH, ROW1], BF16)
    table1 = nc.dram_tensor("table1", [NTAB, ROW1], BF16, addr_space="Shared")
    shard2 = nc.dram_tensor("shard2", [NCH, ROW2], BF16)
    table2 = nc.dram_tensor("table2", [NTAB, ROW2], BF16, addr_space="Shared")
    pool_loc = nc.dram_tensor("pool_loc", [HID, G], F32)
    pool_sh = nc.dram_tensor("pool_sh", [HID, G], F32, addr_space="Shared")

    groups = [tuple(range(a, min(a + cfg.G_CH, CH))) for a in range(0, CH, cfg.G_CH)]

    # SWDGE descriptor-ring pacing (see baseline): probe-read marks gather
    # completion; later gathers dep on the probe to bound outstanding entries.
    gather_fifo = []

    def paced_gather(probe_pool, **kw):
        e = kw["num_idxs"] // 16 + 1
        inst = nc.gpsimd.dma_gather(single_packet=False, **kw)
        gp_t = probe_pool.tile([1, 2], F32, tag="gprobe", name="gprobe")
        rd = nc.vector.tensor_copy(out=gp_t[:], in_=kw["out_ap"][0:1, 0, 0:2])
        tot = sum(x[1] for x in gather_fifo) + e
        while gather_fifo and (tot > 110 or len(gather_fifo) >= 2):
            _, eo, rdo = gather_fifo.pop(0)
            add_dep_helper(inst.ins, rdo.ins, sync=True, reason="swdge ring pacing")
            tot -= eo
        gather_fifo.append((inst, e, rd))
        return inst

    with tile.TileContext(nc) as tc, ExitStack() as ctx:
        cp = ctx.enter_context(tc.tile_pool(name="const", bufs=1))
        dio = ctx.enter_context(tc.tile_pool(name="dio", bufs=3))
        dps = ctx.enter_context(tc.tile_pool(name="dps", bufs=2, space="PSUM"))
        o1p = ctx.enter_context(tc.tile_pool(name="o1p", bufs=1))
        ixp = ctx.enter_context(tc.tile_pool(name="ixp", bufs=2))
        s0pl = ctx.enter_context(tc.tile_pool(name="s0pl", bufs=3))
        gp = ctx.enter_context(tc.tile_pool(name="gp", bufs=2))
        sxp = ctx.enter_context(tc.tile_pool(name="sxp", bufs=6))
        xp = ctx.enter_context(tc.tile_pool(name="xp", bufs=3))
        ups = ctx.enter_context(tc.tile_pool(name="ups", bufs=2, space="PSUM"))
        pps = ctx.enter_context(tc.tile_pool(name="pps", bufs=1, space="PSUM"))
        fin = ctx.enter_context(tc.tile_pool(name="fin", bufs=3))

        io512 = cp.tile([P, G], F32)
        nc.sync.dma_start(out=io512[:], in_=iota512[:])
        r1sb = cp.tile([cfg.IN, 132], F32)
        nc.sync.dma_start(out=r1sb[:], in_=rhs1[:])
        r2sb = cp.tile([2 * HID, HID + 2], F32)
        nc.sync.dma_start(out=r2sb[:], in_=rhs2[:])
        fcsb = cp.tile([HID, 2], F32)
        nc.sync.dma_start(out=fcsb[:], in_=fcW[:])
        icsb = cp.tile([HID, G], F32)
        nc.sync.dma_start(out=icsb[:], in_=invc[:])
        gssb = cp.tile([P, CH], F32)
        nc.sync.dma_start(out=gssb[:], in_=gslot[:])
        idsb = cp.tile([P, P], F32)
        make_identity(nc, idsb[:])
        onesW = cp.tile([P, P], F32)
        nc.vector.memset(onesW[:], 1.0)
        out1 = o1p.tile([P, CH * P], F32)
        AD1 = cp.tile([P, CH * 2 * P], BF16)
        AD2 = cp.tile([P, CH * P], BF16)
        adcol1 = cp.tile([P, 2 * CH], F32)
        adcol2 = cp.tile([P, CH], F32)
        rows1 = cp.tile([P, CH * ROW1], BF16)  # local node rows (self-loop term)
        rows2 = cp.tile([P, CH * ROW2], BF16)
        nc.vector.memset(rows1[:], 0.0)
        nc.vector.memset(rows2[:], 0.0)

        # ---------------- dense 1: rows of table1 + ad columns ----------------
        for t in range(CH):
            xt = dio.tile([P, P], F32, tag="xt")
            nc.sync.dma_start(out=xt[:], in_=xT[:, t * P:(t + 1) * P])
            ps = dps.tile([P, 132], F32, tag="dtmp")
            nc.tensor.matmul(out=ps[:], lhsT=xt[:], rhs=r1sb[:], start=True, stop=True)
            upc = fin.tile([P, 2], F32, tag="upc")
            nc.scalar.activation(out=upc[:], in_=ps[:, 128:130], func=AF.Exp, scale=0.2)
            row = rows1[:, t * ROW1:(t + 1) * ROW1]
            nc.scalar.activation(out=row[:, 0:64], in_=ps[:, 0:64], func=AF.Copy,
                                 scale=upc[:, 0:1])
            nc.scalar.activation(out=row[:, 65:129], in_=ps[:, 64:128], func=AF.Copy,
                                 scale=upc[:, 1:2])
            nc.vector.tensor_copy(out=row[:, 64:65], in_=upc[:, 0:1])
            nc.vector.tensor_copy(out=row[:, 129:130], in_=upc[:, 1:2])
            um = row[:, UO1:UO1 + 12].bitcast(F32)
            nc.vector.tensor_copy(out=um[:, 0:2], in_=ps[:, 128:130])
            nc.vector.tensor_scalar(out=um[:, 2:4], in0=ps[:, 128:130],
                                    scalar1=0.8, scalar2=None, op0=ALU.mult)
            nc.vector.tensor_copy(out=um[:, 4:6], in_=upc[:])
            nc.vector.tensor_copy(out=adcol1[:, 2 * t:2 * t + 2], in_=ps[:, 130:132])
            nc.sync.dma_start(out=shard1[t * P:(t + 1) * P, :], in_=row[:])

        # AD: per (chunk, head) broadcast of ad over the free dim — replicate
        # the ad column along free (tensor_scalar vs ones), PE-transpose, copy.
        def build_ad(adcol_t, ncols, ad_t):
            for i in range(ncols):
                rc = fin.tile([P, P], F32, tag="rc")
                nc.vector.tensor_scalar(out=rc[:], in0=onesW[:],
                                        scalar1=adcol_t[:, i:i + 1], scalar2=None,
                                        op0=ALU.mult)
                vps = dps.tile([P, P], F32, tag="dtmp")
                nc.tensor.transpose(out=vps[:], in_=rc[:], identity=idsb[:])
                nc.scalar.copy(out=ad_t[:, i * P:(i + 1) * P], in_=vps[:])

        build_ad(adcol1, 2 * CH, AD1)

        tc.strict_bb_all_engine_barrier()
        nc.gpsimd.collective_compute(
            "AllGather", ALU.bypass, replica_groups=[R],
            ins=[shard1[:]], outs=[table1[:]])

        # ---------------- edge phase (shared for both layers) ----------------
        def edge_layer(layer, tabA, tabB, ad_t, row_w, uo, nheads, finalize):
            for grp in groups:
                g0, ng = grp[0], len(grp)
                nlo, nhi = ng * SEC_LO, ng * SEC_HI
                glt = ixp.tile([P, nlo // 16], I16, tag="glt")
                nc.sync.dma_start(out=glt[:], in_=gl[:, g0 * SEC_LO // 16:(g0 * SEC_LO + nlo) // 16])
                ght = ixp.tile([P, nhi // 16], I16, tag="ght")
                nc.sync.dma_start(out=ght[:], in_=gh[:, g0 * SEC_HI // 16:(g0 * SEC_HI + nhi) // 16])
                s0t = s0pl.tile([P, ng * T * P], BF16, tag="s0t")
                nc.sync.dma_start(out=s0t[:], in_=s0p[:, g0 * T * P:(g0 + ng) * T * P])
                hgl = gp.tile([P, nlo // P, row_w], BF16, tag="hgl")
                paced_gather(xp, out_ap=hgl[:], in_ap=tabA, idxs_ap=glt[:],
                             num_idxs=nlo, num_idxs_reg=nlo, elem_size=row_w)
                hgh = gp.tile([P, nhi // P, row_w], BF16, tag="hgh")
                paced_gather(xp, out_ap=hgh[:], in_ap=tabB, idxs_ap=ght[:],
                             num_idxs=nhi, num_idxs_reg=nhi, elem_size=row_w)
                for ci, c in enumerate(grp):
                    Us = [ups.tile([P, 65], F32, tag=f"U{h}", name=f"U{h}")
                          for h in range(nheads)]
                    for t in range(T):
                        if t < T_LO:
                            hg_t, tt, nt = hgl, t, T_LO
                        else:
                            hg_t, tt, nt = hgh, t - T_LO, T_HI
                        base = ci * nt + tt
                        s0_t = s0t[:, ((ci * T) + t) * P:((ci * T) + t + 1) * P]
                        uf = hg_t[:, base, uo:uo + 6 * nheads].bitcast(F32)
                        for h in range(nheads):
                            adsl = ad_t[:, (c * nheads + h) * P:(c * nheads + h + 1) * P]
                            Sh = sxp.tile([P, P], BF16, tag=f"Sh{h}")
                            if VARIANT[(layer, h)] == "relu2":
                                Y = sxp.tile([P, P], BF16, tag=f"Y{h}")
                                nc.scalar.activation(out=Y[:], in_=adsl, func=AF.Relu,
                                                     scale=1.0, bias=uf[:, h:h + 1])
                                Gt = sxp.tile([P, P], BF16, tag=f"G{h}")
                                nc.scalar.activation(out=Gt[:], in_=Y[:], func=AF.Exp,
                                                     scale=0.8)
                            else:  # exp1
                                E = sxp.tile([P, P], BF16, tag=f"E{h}")
                                nc.scalar.activation(out=E[:], in_=adsl, func=AF.Exp,
                                                     scale=0.8,
                                                     bias=uf[:, nheads + h:nheads + h + 1])
                                Gt = sxp.tile([P, P], BF16, tag=f"G{h}")
                                nc.vector.tensor_scalar(out=Gt[:], in0=E[:],
                                                        scalar1=1.0, scalar2=None,
                                                        op0=ALU.max)
                            nc.vector.tensor_tensor(out=Sh[:], in0=s0_t, in1=Gt[:],
                                                    op=ALU.mult)
                            nc.tensor.matmul(
                                out=Us[h][:], lhsT=Sh[:],
                                rhs=hg_t[:, base, h * 65:(h + 1) * 65],
                                start=(t == 0), stop=(t == T - 1))
                    finalize(c, Us)

        # self-loop term: wt = e^{0.8 relu(as+ad)}, num += wt*(u'h), den += wt*u'
        def self_w(c, h, rows_t, adcol_t, row_w, uo, nheads):
            uf = rows_t[:, c * row_w + uo:c * row_w + uo + 6 * nheads].bitcast(F32)
            E = fin.tile([P, 1], F32, tag="Ew")
            nc.scalar.activation(out=E[:], in_=adcol_t[:, nheads * c + h:nheads * c + h + 1],
                                 func=AF.Exp, scale=0.8,
                                 bias=uf[:, nheads + h:nheads + h + 1])
            W = fin.tile([P, 1], F32, tag="Ww")
            nc.vector.tensor_scalar(out=W[:], in0=E[:], scalar1=1.0, scalar2=None,
                                    op0=ALU.max)
            wst = fin.tile([P, 1], F32, tag="wst")
            nc.vector.tensor_tensor(out=wst[:], in0=W[:],
                                    in1=uf[:, 2 * nheads + h:2 * nheads + h + 1],
                                    op=ALU.mult)
            numv = fin.tile([P, 64], F32, tag="numv")
            nc.vector.tensor_scalar(out=numv[:], in0=rows_t[:, c * row_w + h * 65:c * row_w + h * 65 + 64],
                                    scalar1=W[:], scalar2=None, op0=ALU.mult)
            return wst, numv

        def fin1(c, Us):
            for h in range(2):
                wst, numv = self_w(c, h, rows1, adcol1, ROW1, UO1, 2)
                un = fin.tile([P, 64], F32, tag="un1")
                nc.vector.tensor_tensor(out=un[:], in0=Us[h][:, 0:64], in1=numv[:],
                                        op=ALU.add)
                den = fin.tile([P, 1], F32, tag="den1")
                nc.vector.tensor_tensor(out=den[:], in0=Us[h][:, 64:65], in1=wst[:],
                                        op=ALU.add)
                rd = fin.tile([P, 1], F32, tag="rd1")
                nc.vector.reciprocal(out=rd[:], in_=den[:])
                nc.vector.tensor_scalar(
                    out=out1[:, c * P + h * 64:c * P + (h + 1) * 64],
                    in0=un[:], scalar1=rd[:], scalar2=0.0,
                    op0=ALU.mult, op1=ALU.max)

        edge_layer(1, table1[0:HALF, :], table1[HALF:NTAB, :], AD1, ROW1, UO1, 2, fin1)

        # ---------------- dense 2 ----------------
        for t in range(CH):
            tp = dps.tile([P, P], F32, tag="dtmp")
            nc.tensor.transpose(out=tp[:], in_=out1[:, t * P:(t + 1) * P], identity=idsb[:])
            h1T = dio.tile([P, P], F32, tag="h1T")
            nc.scalar.copy(out=h1T[:], in_=tp[:])
            ps = dps.tile([P, HID + 2], F32, tag="dtmp")
            nc.tensor.matmul(out=ps[:], lhsT=h1T[:], rhs=r2sb[:], start=True, stop=True)
            upc = fin.tile([P, 1], F32, tag="upc2")
            nc.scalar.activation(out=upc[:], in_=ps[:, 64:65], func=AF.Exp, scale=0.2)
            row = rows2[:, t * ROW2:(t + 1) * ROW2]
            nc.scalar.activation(out=row[:, 0:64], in_=ps[:, 0:64], func=AF.Copy,
                                 scale=upc[:, 0:1])
            nc.vector.tensor_copy(out=row[:, 64:65], in_=upc[:, 0:1])
            um = row[:, UO2:UO2 + 6].bitcast(F32)
            nc.vector.tensor_copy(out=um[:, 0:1], in_=ps[:, 64:65])
            nc.vector.tensor_scalar(out=um[:, 1:2], in0=ps[:, 64:65],
                                    scalar1=0.8, scalar2=None, op0=ALU.mult)
            nc.vector.tensor_copy(out=um[:, 2:3], in_=upc[:])
            nc.vector.tensor_copy(out=adcol2[:, t:t + 1], in_=ps[:, 65:66])
            nc.sync.dma_start(out=shard2[t * P:(t + 1) * P, :], in_=row[:])

        build_ad(adcol2, CH, AD2)

        tc.strict_bb_all_engine_barrier()
        nc.gpsimd.collective_compute(
            "AllGather", ALU.bypass, replica_groups=[R],
            ins=[shard2[:]], outs=[table2[:]])

        # ---------------- edge layer 2 + pooling ----------------
        plT = pps.tile([HID, G], F32)

        def fin2(c, Us):
            wst, numv = self_w(c, 0, rows2, adcol2, ROW2, UO2, 1)
            un = fin.tile([P, 64], F32, tag="un2")
            nc.vector.tensor_tensor(out=un[:], in0=Us[0][:, 0:64], in1=numv[:],
                                    op=ALU.add)
            den = fin.tile([P, 1], F32, tag="den2")
            nc.vector.tensor_tensor(out=den[:], in0=Us[0][:, 64:65], in1=wst[:],
                                    op=ALU.add)
            rd = fin.tile([P, 1], F32, tag="rd2")
            nc.vector.reciprocal(out=rd[:], in_=den[:])
            o2 = fin.tile([P, HID], F32, tag="o2")
            nc.vector.tensor_scalar(out=o2[:], in0=un[:],
                                    scalar1=rd[:], scalar2=0.0,
                                    op0=ALU.mult, op1=ALU.max)
            sg = fin.tile([P, G], F32, tag="sg")
            nc.vector.tensor_scalar(out=sg[:], in0=io512[:],
                                    scalar1=gssb[:, c:c + 1], scalar2=None,
                                    op0=ALU.is_equal)
            nc.tensor.matmul(out=plT[:], lhsT=o2[:], rhs=sg[:],
                             start=(c == 0), stop=(c == CH - 1))

        edge_layer(2, table2[0:HALF, :], table2[HALF:NTAB, :], AD2, ROW2, UO2, 1, fin2)

        plsb = fin.tile([HID, G], F32)
        nc.vector.tensor_copy(out=plsb[:], in_=plT[:])
        nc.sync.dma_start(out=pool_loc[:], in_=plsb[:])
        tc.strict_bb_all_engine_barrier()
        nc.gpsimd.collective_compute(
            "AllReduce", ALU.add, replica_groups=[R],
            ins=[pool_loc[:]], outs=[pool_sh[:]])
        plr = fin.tile([HID, G], F32)
        nc.sync.dma_start(out=plr[:], in_=pool_sh[:])
        nc.vector.tensor_tensor(out=plr[:], in0=plr[:], in1=icsb[:], op=ALU.mult)
        for gt in range(max(1, G // P)):
            gw = min(P, G - gt * P)
            lg = dps.tile([P, 2], F32, tag="dtmp")
            nc.tensor.matmul(out=lg[:gw], lhsT=plr[:, gt * P:gt * P + gw], rhs=fcsb[:],
                             start=True, stop=True)
            mx = fin.tile([P, 1], F32, tag="mx")
            nc.vector.tensor_reduce(out=mx[:gw], in_=lg[:gw], op=ALU.max,
                                    axis=mybir.AxisListType.X)
            t1 = fin.tile([P, 2], F32, tag="t1")
            nc.vector.tensor_scalar(out=t1[:gw], in0=lg[:gw], scalar1=mx[:gw],
                                    scalar2=None, op0=ALU.subtract)
            ex = fin.tile([P, 2], F32, tag="ex")
            es = fin.tile([P, 1], F32, tag="es")
            nc.scalar.activation(out=ex[:gw], in_=t1[:gw], func=AF.Exp, accum_out=es[:gw])
            ln = fin.tile([P, 1], F32, tag="ln")
            nc.scalar.activation(out=ln[:gw], in_=es[:gw], func=AF.Ln)
            lsm = fin.tile([P, 2], F32, tag="lsm")
            nc.vector.tensor_scalar(out=lsm[:gw], in0=t1[:gw], scalar1=ln[:gw],
                                    scalar2=None, op0=ALU.subtract)
            nc.sync.dma_start(out=out_lg[gt * P:gt * P + gw, :], in_=lsm[:gw])

    nc.compile()
    return nc


# ------------------------------------------------------------------ entry

LAST_EXEC_NS = None

def kernel(x, edge_index, batch, W1, a_src1, a_dst1, b1, W2, a_src2, a_dst2, b2,
           fcW, fcb):
    x = np.asarray(x, np.float32)
    edge_index = np.asarray(edge_index, np.int64)
    batch = np.asarray(batch, np.int64)
    for b in (b1, b2, fcb):
        assert np.abs(np.asarray(b)).max() == 0.0, "nonzero bias unsupported"
    cfg = plan_cfg(N=x.shape[0], E0=edge_index.shape[1], G=512)
    in_maps = prep_inputs(cfg, x, edge_index, batch,
                          np.asarray(W1, np.float32), np.asarray(a_src1, np.float32),
                          np.asarray(a_dst1, np.float32), np.asarray(W2, np.float32),
                          np.asarray(a_src2, np.float32), np.asarray(a_dst2, np.float32),
                          np.asarray(fcW, np.float32))
    nc = build_nc(cfg)
    trace = os.environ.get("KERNEL_TRACE") == "1"
    res = run_bass_kernel_spmd(nc, in_maps, list(range(cfg.CORES)), trace=trace)
    global LAST_EXEC_NS
    LAST_EXEC_NS = res.exec_time_ns
    if trace:
        print(f"HW exec time: {res.exec_time_ns} ns "
              f"(mean {res.mean_exec_time_ns} ns)")
    return np.asarray(res.results[0]["out_lg"], np.float32)


# revision 15
# speedup vs baseline: 2.0189x; 1.1300x over previous
"""GAT classifier on 8 trn2 NeuronCores (Bass/Tile).

Sharding: 1D node partition (6250 nodes/core, padded to 6272 = 49 chunks of
128 slots); edges assigned to the core owning their dst node, grouped by dst
chunk. Host balances node->chunk assignment on per-half in-degree so the
padded per-(chunk,half) edge-section size is minimal. Self-loops are applied
locally in the finalize step (operands are chunk-local), not gathered.

Per 128-edge tile the segmented softmax + weighted aggregation is one
PSUM-accumulated "selection matmul": U[j, :] += Sh^T @ [u'h_rows | u'],
Sh[e, j] = S0[e, j] * wt[e, j], where S0[e, j] = (slot_e == j) is a STATIC
0/1 mask streamed from the host, and the weight uses the factorization
  exp(leaky_relu(as + ad)) / e^{0.2(as+ad)} = e^{0.8 relu(as + ad)} = wt
(the e^{0.2 as} factor is premultiplied into the source row as u' = e^{0.2 as},
and e^{0.2 ad} cancels in the softmax ratio). wt is built per tile by the
SCALAR engine from a per-chunk broadcast tile AD[e, j] = ad_j and a
per-partition bias as_e carried in the gathered row (fp32 inside the bf16
row), then masked with one DVE tensor_tensor. Node tables are bf16 (512B
rows L1, 256B rows L2), AllGathered between layers; pooling partials are
AllReduced.
"""
import math
import os
import sys
from contextlib import ExitStack
from dataclasses import dataclass

import numpy as np
import ml_dtypes

for _p in ("/opt/trn_rl_repo", "/root/.axon_site/_ro/trn_rl_repo"):
    if os.path.isdir(_p) and _p not in sys.path:
        sys.path.insert(0, _p)

import concourse.bacc as bacc
import concourse.bass as bass
import concourse.mybir as mybir
import concourse.tile as tile
from concourse.tile import add_dep_helper
from concourse.bass_utils import run_bass_kernel_spmd
from concourse.masks import make_identity

P = 128
AF = mybir.ActivationFunctionType
ALU = mybir.AluOpType
F32 = mybir.dt.float32
BF16 = mybir.dt.bfloat16
I16 = mybir.dt.int16
BF = ml_dtypes.bfloat16

ROW1 = 256  # bf16: [u'0*h0(64) u'0 u'1*h1(64) u'1 pad(2) f32:{as0 as1 as08_0 as08_1 u'0 u'1}]
ROW2 = 128  # bf16: [u'*h2(64) u' pad(1) f32:{as as08 u'}]
UO1 = 132   # bf16-elem offset of the fp32 block
UO2 = 66

# weight-build variant per (layer, head): "relu2" = 2 scalar acts + 1 DVE TT;
# "exp1" = 1 scalar act + 1 DVE TS + 1 DVE TT. Chosen to balance Scalar vs DVE.
VARIANT = {(1, 0): "relu2", (1, 1): "exp1", (2, 0): "exp1"}


@dataclass
class Cfg:
    N: int = 50000
    E0: int = 800000
    IN: int = 128
    HID: int = 64
    G: int = 512
    CORES: int = 8
    NPC: int = 0
    CH: int = 0
    SEC_LO: int = 0
    SEC_HI: int = 0
    T: int = 0
    T_LO: int = 0
    T_HI: int = 0
    G_CH: int = 2

    @property
    def NCH(self):  # padded per-core node count
        return self.CH * P

    @property
    def HALF(self):  # table rows of cores 0-3 (lo half for int16 indices)
        return (self.CORES // 2) * self.NCH

    @property
    def NTAB(self):
        return self.CORES * self.NCH


def plan_cfg(N, E0, G, CORES=8):
    c = Cfg(N=N, E0=E0, G=G, CORES=CORES)
    assert N % CORES == 0
    c.NPC = N // CORES
    c.CH = math.ceil(c.NPC / P)
    assert c.HALF <= 32767 and (c.NTAB - c.HALF) <= 32768
    return c


# ----------------------------------------------------------------- host prep

def balance_nodes(cfg, src, dst):
    """Assign nodes to (chunk, slot) per core, balancing per-half in-degree
    across chunks. Returns perm_rows: global node id -> global table row."""
    N, NPC, NCH, CH = cfg.N, cfg.NPC, cfg.NCH, cfg.CH
    deg = np.zeros((N, 2), np.int64)
    np.add.at(deg, (dst, ((src // NPC) >= (cfg.CORES // 2)).astype(np.int64)), 1)
    perm_rows = np.empty(N, np.int64)
    for c in range(cfg.CORES):
        dl = deg[c * NPC:(c + 1) * NPC].astype(np.float64)
        order = np.argsort(-(dl[:, 0] + dl[:, 1]), kind="stable")
        loads = np.zeros((CH, 2))
        cnts = np.zeros(CH, np.int64)
        for l in order:
            cand = np.maximum(loads[:, 0] + dl[l, 0], loads[:, 1] + dl[l, 1])
            cand[cnts >= P] = np.inf
            k = int(np.argmin(cand))
            perm_rows[c * NPC + l] = c * NCH + k * P + cnts[k]
            loads[k, 0] += dl[l, 0]
            loads[k, 1] += dl[l, 1]
            cnts[k] += 1
    return perm_rows


def prep_edges(cfg, src, dst, perm_rows):
    """Per-core edge index arrays + static S0 mask tiles. Fills cfg.SEC_*/T_*."""
    CH, NPC, NCH, HALF = cfg.CH, cfg.NPC, cfg.NCH, cfg.HALF
    per_core = []
    maxlo = maxhi = 0
    for c in range(cfg.CORES):
        m = (dst // NPC) == c
        srow = perm_rows[src[m]]
        dloc = perm_rows[dst[m]] - c * NCH
        chunk = dloc >> 7
        half = (srow >= HALF).astype(np.int64)
        order = np.lexsort((srow, half, chunk))
        srow, dloc, chunk, half = srow[order], dloc[order], chunk[order], half[order]
        key = chunk * 2 + half
        cnt = np.bincount(key, minlength=CH * 2).reshape(CH, 2)
        maxlo = max(maxlo, int(cnt[:, 0].max()))
        maxhi = max(maxhi, int(cnt[:, 1].max()))
        per_core.append((srow, dloc, cnt))
    cfg.SEC_LO = ((maxlo + 127) & ~127) or P
    cfg.SEC_HI = ((maxhi + 127) & ~127) or P
    cfg.T_LO = cfg.SEC_LO // P
    cfg.T_HI = cfg.SEC_HI // P
    cfg.T = cfg.T_LO + cfg.T_HI
    EC = cfg.SEC_LO + cfg.SEC_HI

    def wrap16(a):  # idx i -> [i % 16, i // 16], replicated over 8 groups
        w = a.reshape(-1, 16).T.copy()
        return np.tile(w, (8, 1)).astype(np.int16)

    jj = np.arange(P, dtype=np.int64)
    out = []
    for c in range(cfg.CORES):
        srow, dloc, cnt = per_core[c]
        gl = np.zeros((CH, cfg.SEC_LO), np.int16)
        gh = np.zeros((CH, cfg.SEC_HI), np.int16)
        sl = np.full((CH, EC), 512, np.int64)
        ofs = np.zeros(CH * 2 + 1, np.int64)
        np.cumsum(cnt.reshape(-1), out=ofs[1:])
        for k in range(CH):
            nlo, nhi = int(cnt[k, 0]), int(cnt[k, 1])
            a = ofs[2 * k]
            gl[k, :nlo] = srow[a:a + nlo]
            sl[k, :nlo] = dloc[a:a + nlo] & 127
            b = ofs[2 * k + 1]
            gh[k, :nhi] = srow[b:b + nhi] - cfg.HALF
            sl[k, cfg.SEC_LO:cfg.SEC_LO + nhi] = dloc[b:b + nhi] & 127
        # S0 mask tiles: [e, tile*128 + j] = (slot == j), tile-major
        s0 = (sl.reshape(CH * cfg.T, P)[:, :, None] == jj[None, None, :])
        s0 = np.ascontiguousarray(
            s0.transpose(1, 0, 2).reshape(P, CH * cfg.T * P)).astype(BF)
        out.append(dict(gl=wrap16(gl), gh=wrap16(gh), s0=s0))
    return out


def prep_inputs(cfg, x, edge_index, batch, W1, a_src1, a_dst1, W2, a_src2, a_dst2, fcW):
    N, CORES, NPC, NCH, CH = cfg.N, cfg.CORES, cfg.NPC, cfg.NCH, cfg.CH
    # self-loops are NOT in the gathered edge lists — they are applied
    # locally in the finalize step (all their operands are chunk-local)
    src = edge_index[0].astype(np.int64)
    dst = edge_index[1].astype(np.int64)
    perm_rows = balance_nodes(cfg, src, dst)
    edges = prep_edges(cfg, src, dst, perm_rows)

    H = 2
    HID = cfg.HID
    rhs1 = np.zeros((cfg.IN, 132), np.float32)
    rhs1[:, :H * HID] = W1
    for h in range(H):
        rhs1[:, H * HID + h] = W1[:, h * HID:(h + 1) * HID] @ a_src1[h]
        rhs1[:, H * HID + 2 + h] = W1[:, h * HID:(h + 1) * HID] @ a_dst1[h]
    rhs2 = np.zeros((H * HID, HID + 2), np.float32)
    rhs2[:, :HID] = W2
    rhs2[:, HID] = W2 @ a_src2[0]
    rhs2[:, HID + 1] = W2 @ a_dst2[0]

    iota512 = np.tile(np.arange(cfg.G, dtype=np.float32), (P, 1))
    cnt = np.bincount(batch, minlength=cfg.G).astype(np.float32)
    invc_b = np.tile(1.0 / np.maximum(cnt, 1.0), (HID, 1)).astype(np.float32)

    xT = np.zeros((cfg.IN, CORES * NCH), np.float32)
    gsl = np.full((CORES, NCH), 999.0, np.float32)
    for c in range(CORES):
        nodes = np.arange(c * NPC, (c + 1) * NPC)
        rows = perm_rows[nodes] - c * NCH
        xT[:, c * NCH + rows] = x[nodes].T
        gsl[c, rows] = batch[nodes]

    in_maps = []
    for c in range(CORES):
        in_maps.append(dict(
            xT=np.ascontiguousarray(xT[:, c * NCH:(c + 1) * NCH]),
            rhs1=rhs1, rhs2=rhs2, fcW=fcW.astype(np.float32),
            iota512=iota512, invc=invc_b,
            gslot=gsl[c].reshape(CH, P).T.copy(),
            **edges[c],
        ))
    return in_maps


# -------------------------------------------------------------- bass builder

def build_nc(cfg):
    CH, T, T_LO, T_HI = cfg.CH, cfg.T, cfg.T_LO, cfg.T_HI
    SEC_LO, SEC_HI = cfg.SEC_LO, cfg.SEC_HI
    HID, G, NCH, HALF, NTAB = cfg.HID, cfg.G, cfg.NCH, cfg.HALF, cfg.NTAB
    R = list(range(cfg.CORES))

    nc = bacc.Bacc()
    pi = lambda n, s, d=F32: nc.declare_dram_parameter(n, s, d, isOutput=False)
    xT = pi("xT", [cfg.IN, NCH])
    rhs1 = pi("rhs1", [cfg.IN, 132])
    rhs2 = pi("rhs2", [2 * HID, HID + 2])
    fcW = pi("fcW", [HID, 2])
    iota512 = pi("iota512", [P, G])
    invc = pi("invc", [HID, G])
    gslot = pi("gslot", [P, CH])
    gl = pi("gl", [P, CH * SEC_LO // 16], I16)
    gh = pi("gh", [P, CH * SEC_HI // 16], I16)
    s0p = pi("s0", [P, CH * T * P], BF16)
    out_lg = nc.declare_dram_parameter("out_lg", [G, 2], F32, isOutput=True)

    shard1 = nc.dram_tensor("shard1", [NCH, ROW1], BF16)
    table1 = nc.dram_tensor("table1", [NTAB, ROW1], BF16, addr_space="Shared")
    shard2 = nc.dram_tensor("shard2", [NCH, ROW2], BF16)
    table2 = nc.dram_tensor("table2", [NTAB, ROW2], BF16, addr_space="Shared")
    pool_loc = nc.dram_tensor("pool_loc", [HID, G], F32)
    pool_sh = nc.dram_tensor("pool_sh", [HID, G], F32, addr_space="Shared")

    groups = [tuple(range(a, min(a + cfg.G_CH, CH))) for a in range(0, CH, cfg.G_CH)]

    # SWDGE descriptor-ring pacing (see baseline): a probe marks gather
    # completion; later gathers dep on the probe to bound outstanding entries.
    # The probe is a tiny SYNC-engine DMA (not a DVE op): the sync queue is
    # nearly empty, so the probe fires as soon as the gather's DMA lands
    # instead of queueing behind a group's worth of DVE tile ops.
    gather_fifo = []

    def paced_gather(probe_pool, **kw):
        e = kw["num_idxs"] // 16 + 1
        inst = nc.gpsimd.dma_gather(single_packet=False, **kw)
        gp_t = probe_pool.tile([1, 2], BF16, tag="gprobe", name="gprobe")
        rd = nc.sync.dma_start(out=gp_t[:], in_=kw["out_ap"][0:1, 0, 0:2])
        tot = sum(x[1] for x in gather_fifo) + e
        while gather_fifo and (tot > 110 or len(gather_fifo) >= 2):
            _, eo, rdo = gather_fifo.pop(0)
            add_dep_helper(inst.ins, rdo.ins, sync=True, reason="swdge ring pacing")
            tot -= eo
        gather_fifo.append((inst, e, rd))
        return inst

    with tile.TileContext(nc) as tc, ExitStack() as ctx:
        cp = ctx.enter_context(tc.tile_pool(name="const", bufs=1))
        dio = ctx.enter_context(tc.tile_pool(name="dio", bufs=3))
        dps = ctx.enter_context(tc.tile_pool(name="dps", bufs=2, space="PSUM"))
        o1p = ctx.enter_context(tc.tile_pool(name="o1p", bufs=1))
        ixp = ctx.enter_context(tc.tile_pool(name="ixp", bufs=2))
        s0pl = ctx.enter_context(tc.tile_pool(name="s0pl", bufs=3))
        gp = ctx.enter_context(tc.tile_pool(name="gp", bufs=2))
        sxp = ctx.enter_context(tc.tile_pool(name="sxp", bufs=6))
        xp = ctx.enter_context(tc.tile_pool(name="xp", bufs=3))
        ups = ctx.enter_context(tc.tile_pool(name="ups", bufs=2, space="PSUM"))
        pps = ctx.enter_context(tc.tile_pool(name="pps", bufs=1, space="PSUM"))
        fin = ctx.enter_context(tc.tile_pool(name="fin", bufs=3))

        io512 = cp.tile([P, G], F32)
        nc.sync.dma_start(out=io512[:], in_=iota512[:])
        r1sb = cp.tile([cfg.IN, 132], F32)
        nc.sync.dma_start(out=r1sb[:], in_=rhs1[:])
        r2sb = cp.tile([2 * HID, HID + 2], F32)
        nc.sync.dma_start(out=r2sb[:], in_=rhs2[:])
        fcsb = cp.tile([HID, 2], F32)
        nc.sync.dma_start(out=fcsb[:], in_=fcW[:])
        icsb = cp.tile([HID, G], F32)
        nc.sync.dma_start(out=icsb[:], in_=invc[:])
        gssb = cp.tile([P, CH], F32)
        nc.sync.dma_start(out=gssb[:], in_=gslot[:])
        idsb = cp.tile([P, P], F32)
        make_identity(nc, idsb[:])
        onesW = cp.tile([P, P], F32)
        nc.vector.memset(onesW[:], 1.0)
        out1 = o1p.tile([P, CH * P], F32)
        AD1 = cp.tile([P, CH * 2 * P], BF16)
        AD2 = cp.tile([P, CH * P], BF16)
        adcol1 = cp.tile([P, 2 * CH], F32)
        adcol2 = cp.tile([P, CH], F32)
        rows1 = cp.tile([P, CH * ROW1], BF16)  # local node rows (self-loop term)
        rows2 = cp.tile([P, CH * ROW2], BF16)
        nc.vector.memset(rows1[:], 0.0)
        nc.vector.memset(rows2[:], 0.0)

        # ---------------- dense 1: rows of table1 + ad columns ----------------
        for t in range(CH):
            xt = dio.tile([P, P], F32, tag="xt")
            nc.sync.dma_start(out=xt[:], in_=xT[:, t * P:(t + 1) * P])
            ps = dps.tile([P, 132], F32, tag="dtmp")
            nc.tensor.matmul(out=ps[:], lhsT=xt[:], rhs=r1sb[:], start=True, stop=True)
            upc = fin.tile([P, 2], F32, tag="upc")
            nc.scalar.activation(out=upc[:], in_=ps[:, 128:130], func=AF.Exp, scale=0.2)
            row = rows1[:, t * ROW1:(t + 1) * ROW1]
            nc.scalar.activation(out=row[:, 0:64], in_=ps[:, 0:64], func=AF.Copy,
                                 scale=upc[:, 0:1])
            nc.scalar.activation(out=row[:, 65:129], in_=ps[:, 64:128], func=AF.Copy,
                                 scale=upc[:, 1:2])
            nc.vector.tensor_copy(out=row[:, 64:65], in_=upc[:, 0:1])
            nc.vector.tensor_copy(out=row[:, 129:130], in_=upc[:, 1:2])
            um = row[:, UO1:UO1 + 12].bitcast(F32)
            nc.vector.tensor_copy(out=um[:, 0:2], in_=ps[:, 128:130])
            nc.vector.tensor_scalar(out=um[:, 2:4], in0=ps[:, 128:130],
                                    scalar1=0.8, scalar2=None, op0=ALU.mult)
            nc.vector.tensor_copy(out=um[:, 4:6], in_=upc[:])
            nc.vector.tensor_copy(out=adcol1[:, 2 * t:2 * t + 2], in_=ps[:, 130:132])
            nc.sync.dma_start(out=shard1[t * P:(t + 1) * P, :], in_=row[:])

        # AD: per (chunk, head) broadcast of ad over the free dim — replicate
        # the ad column along free (tensor_scalar vs ones), PE-transpose, copy.
        def build_ad(adcol_t, ncols, ad_t):
            for i in range(ncols):
                rc = fin.tile([P, P], F32, tag="rc")
                nc.vector.tensor_scalar(out=rc[:], in0=onesW[:],
                                        scalar1=adcol_t[:, i:i + 1], scalar2=None,
                                        op0=ALU.mult)
                vps = dps.tile([P, P], F32, tag="dtmp")
                nc.tensor.transpose(out=vps[:], in_=rc[:], identity=idsb[:])
                nc.scalar.copy(out=ad_t[:, i * P:(i + 1) * P], in_=vps[:])

        build_ad(adcol1, 2 * CH, AD1)

        tc.strict_bb_all_engine_barrier()
        nc.gpsimd.collective_compute(
            "AllGather", ALU.bypass, replica_groups=[R],
            ins=[shard1[:]], outs=[table1[:]])

        # ---------------- edge phase (shared for both layers) ----------------
        def edge_layer(layer, tabA, tabB, ad_t, row_w, uo, nheads, finalize):
            for grp in groups:
                g0, ng = grp[0], len(grp)
                nlo, nhi = ng * SEC_LO, ng * SEC_HI
                glt = ixp.tile([P, nlo // 16], I16, tag="glt")
                nc.sync.dma_start(out=glt[:], in_=gl[:, g0 * SEC_LO // 16:(g0 * SEC_LO + nlo) // 16])
                ght = ixp.tile([P, nhi // 16], I16, tag="ght")
                nc.sync.dma_start(out=ght[:], in_=gh[:, g0 * SEC_HI // 16:(g0 * SEC_HI + nhi) // 16])
                s0t = s0pl.tile([P, ng * T * P], BF16, tag="s0t")
                nc.sync.dma_start(out=s0t[:], in_=s0p[:, g0 * T * P:(g0 + ng) * T * P])
                hgl = gp.tile([P, nlo // P, row_w], BF16, tag="hgl")
                paced_gather(xp, out_ap=hgl[:], in_ap=tabA, idxs_ap=glt[:],
                             num_idxs=nlo, num_idxs_reg=nlo, elem_size=row_w)
                hgh = gp.tile([P, nhi // P, row_w], BF16, tag="hgh")
                paced_gather(xp, out_ap=hgh[:], in_ap=tabB, idxs_ap=ght[:],
                             num_idxs=nhi, num_idxs_reg=nhi, elem_size=row_w)
                for ci, c in enumerate(grp):
                    Us = [ups.tile([P, 65], F32, tag=f"U{h}", name=f"U{h}")
                          for h in range(nheads)]
                    for t in range(T):
                        if t < T_LO:
                            hg_t, tt, nt = hgl, t, T_LO
                        else:
                            hg_t, tt, nt = hgh, t - T_LO, T_HI
                        base = ci * nt + tt
                        s0_t = s0t[:, ((ci * T) + t) * P:((ci * T) + t + 1) * P]
                        uf = hg_t[:, base, uo:uo + 6 * nheads].bitcast(F32)
                        for h in range(nheads):
                            adsl = ad_t[:, (c * nheads + h) * P:(c * nheads + h + 1) * P]
                            Sh = sxp.tile([P, P], BF16, tag=f"Sh{h}")
                            if VARIANT[(layer, h)] == "relu2":
                                Y = sxp.tile([P, P], BF16, tag=f"Y{h}")
                                nc.scalar.activation(out=Y[:], in_=adsl, func=AF.Relu,
                                                     scale=1.0, bias=uf[:, h:h + 1])
                                Gt = sxp.tile([P, P], BF16, tag=f"G{h}")
                                nc.scalar.activation(out=Gt[:], in_=Y[:], func=AF.Exp,
                                                     scale=0.8)
                            else:  # exp1
                                E = sxp.tile([P, P], BF16, tag=f"E{h}")
                                nc.scalar.activation(out=E[:], in_=adsl, func=AF.Exp,
                                                     scale=0.8,
                                                     bias=uf[:, nheads + h:nheads + h + 1])
                                Gt = sxp.tile([P, P], BF16, tag=f"G{h}")
                                nc.vector.tensor_scalar(out=Gt[:], in0=E[:],
                                                        scalar1=1.0, scalar2=None,
                                                        op0=ALU.max)
                            nc.vector.tensor_tensor(out=Sh[:], in0=s0_t, in1=Gt[:],
                                                    op=ALU.mult)
                            nc.tensor.matmul(
                                out=Us[h][:], lhsT=Sh[:],
                                rhs=hg_t[:, base, h * 65:(h + 1) * 65],
                                start=(t == 0), stop=(t == T - 1))
                    finalize(c, Us)

        # self-loop term: wt = e^{0.8 relu(as+ad)}, num += wt*(u'h), den += wt*u'
        def self_w(c, h, rows_t, adcol_t, row_w, uo, nheads):
            uf = rows_t[:, c * row_w + uo:c * row_w + uo + 6 * nheads].bitcast(F32)
            E = fin.tile([P, 1], F32, tag="Ew")
            nc.scalar.activation(out=E[:], in_=adcol_t[:, nheads * c + h:nheads * c + h + 1],
                                 func=AF.Exp, scale=0.8,
                                 bias=uf[:, nheads + h:nheads + h + 1])
            W = fin.tile([P, 1], F32, tag="Ww")
            nc.vector.tensor_scalar(out=W[:], in0=E[:], scalar1=1.0, scalar2=None,
                                    op0=ALU.max)
            wst = fin.tile([P, 1], F32, tag="wst")
            nc.vector.tensor_tensor(out=wst[:], in0=W[:],
                                    in1=uf[:, 2 * nheads + h:2 * nheads + h + 1],
                                    op=ALU.mult)
            numv = fin.tile([P, 64], F32, tag="numv")
            nc.vector.tensor_scalar(out=numv[:], in0=rows_t[:, c * row_w + h * 65:c * row_w + h * 65 + 64],
                                    scalar1=W[:], scalar2=None, op0=ALU.mult)
            return wst, numv

        def fin1(c, Us):
            for h in range(2):
                wst, numv = self_w(c, h, rows1, adcol1, ROW1, UO1, 2)
                un = fin.tile([P, 64], F32, tag="un1")
                nc.vector.tensor_tensor(out=un[:], in0=Us[h][:, 0:64], in1=numv[:],
                                        op=ALU.add)
                den = fin.tile([P, 1], F32, tag="den1")
                nc.vector.tensor_tensor(out=den[:], in0=Us[h][:, 64:65], in1=wst[:],
                                        op=ALU.add)
                rd = fin.tile([P, 1], F32, tag="rd1")
                nc.vector.reciprocal(out=rd[:], in_=den[:])
                nc.vector.tensor_scalar(
                    out=out1[:, c * P + h * 64:c * P + (h + 1) * 64],
                    in0=un[:], scalar1=rd[:], scalar2=0.0,
                    op0=ALU.mult, op1=ALU.max)

        edge_layer(1, table1[0:HALF, :], table1[HALF:NTAB, :], AD1, ROW1, UO1, 2, fin1)

        # ---------------- dense 2 ----------------
        for t in range(CH):
            tp = dps.tile([P, P], F32, tag="dtmp")
            nc.tensor.transpose(out=tp[:], in_=out1[:, t * P:(t + 1) * P], identity=idsb[:])
            h1T = dio.tile([P, P], F32, tag="h1T")
            nc.scalar.copy(out=h1T[:], in_=tp[:])
            ps = dps.tile([P, HID + 2], F32, tag="dtmp")
            nc.tensor.matmul(out=ps[:], lhsT=h1T[:], rhs=r2sb[:], start=True, stop=True)
            upc = fin.tile([P, 1], F32, tag="upc2")
            nc.scalar.activation(out=upc[:], in_=ps[:, 64:65], func=AF.Exp, scale=0.2)
            row = rows2[:, t * ROW2:(t + 1) * ROW2]
            nc.scalar.activation(out=row[:, 0:64], in_=ps[:, 0:64], func=AF.Copy,
                                 scale=upc[:, 0:1])
            nc.vector.tensor_copy(out=row[:, 64:65], in_=upc[:, 0:1])
            um = row[:, UO2:UO2 + 6].bitcast(F32)
            nc.vector.tensor_copy(out=um[:, 0:1], in_=ps[:, 64:65])
            nc.vector.tensor_scalar(out=um[:, 1:2], in0=ps[:, 64:65],
                                    scalar1=0.8, scalar2=None, op0=ALU.mult)
            nc.vector.tensor_copy(out=um[:, 2:3], in_=upc[:])
            nc.vector.tensor_copy(out=adcol2[:, t:t + 1], in_=ps[:, 65:66])
            nc.sync.dma_start(out=shard2[t * P:(t + 1) * P, :], in_=row[:])

        build_ad(adcol2, CH, AD2)

        tc.strict_bb_all_engine_barrier()
        nc.gpsimd.collective_compute(
            "AllGather", ALU.bypass, replica_groups=[R],
            ins=[shard2[:]], outs=[table2[:]])

        # ---------------- edge layer 2 + pooling ----------------
        plT = pps.tile([HID, G], F32)

        def fin2(c, Us):
            wst, numv = self_w(c, 0, rows2, adcol2, ROW2, UO2, 1)
            un = fin.tile([P, 64], F32, tag="un2")
            nc.vector.tensor_tensor(out=un[:], in0=Us[0][:, 0:64], in1=numv[:],
                                    op=ALU.add)
            den = fin.tile([P, 1], F32, tag="den2")
            nc.vector.tensor_tensor(out=den[:], in0=Us[0][:, 64:65], in1=wst[:],
                                    op=ALU.add)
            rd = fin.tile([P, 1], F32, tag="rd2")
            nc.vector.reciprocal(out=rd[:], in_=den[:])
            o2 = fin.tile([P, HID], F32, tag="o2")
            nc.vector.tensor_scalar(out=o2[:], in0=un[:],
                                    scalar1=rd[:], scalar2=0.0,
                                    op0=ALU.mult, op1=ALU.max)
            sg = fin.tile([P, G], F32, tag="sg")
            nc.vector.tensor_scalar(out=sg[:], in0=io512[:],
                                    scalar1=gssb[:, c:c + 1], scalar2=None,
                                    op0=ALU.is_equal)
            nc.tensor.matmul(out=plT[:], lhsT=o2[:], rhs=sg[:],
                             start=(c == 0), stop=(c == CH - 1))

        edge_layer(2, table2[0:HALF, :], table2[HALF:NTAB, :], AD2, ROW2, UO2, 1, fin2)

        plsb = fin.tile([HID, G], F32)
        nc.vector.tensor_copy(out=plsb[:], in_=plT[:])
        nc.sync.dma_start(out=pool_loc[:], in_=plsb[:])
        tc.strict_bb_all_engine_barrier()
        nc.gpsimd.collective_compute(
            "AllReduce", ALU.add, replica_groups=[R],
            ins=[pool_loc[:]], outs=[pool_sh[:]])
        plr = fin.tile([HID, G], F32)
        nc.sync.dma_start(out=plr[:], in_=pool_sh[:])
        nc.vector.tensor_tensor(out=plr[:], in0=plr[:], in1=icsb[:], op=ALU.mult)
        for gt in range(max(1, G // P)):
            gw = min(P, G - gt * P)
            lg = dps.tile([P, 2], F32, tag="dtmp")
            nc.tensor.matmul(out=lg[:gw], lhsT=plr[:, gt * P:gt * P + gw], rhs=fcsb[:],
                             start=True, stop=True)
            mx = fin.tile([P, 1], F32, tag="mx")
            nc.vector.tensor_reduce(out=mx[:gw], in_=lg[:gw], op=ALU.max,
                                    axis=mybir.AxisListType.X)
            t1 = fin.tile([P, 2], F32, tag="t1")
            nc.vector.tensor_scalar(out=t1[:gw], in0=lg[:gw], scalar1=mx[:gw],
                                    scalar2=None, op0=ALU.subtract)
            ex = fin.tile([P, 2], F32, tag="ex")
            es = fin.tile([P, 1], F32, tag="es")
            nc.scalar.activation(out=ex[:gw], in_=t1[:gw], func=AF.Exp, accum_out=es[:gw])
            ln = fin.tile([P, 1], F32, tag="ln")
            nc.scalar.activation(out=ln[:gw], in_=es[:gw], func=AF.Ln)
            lsm = fin.tile([P, 2], F32, tag="lsm")
            nc.vector.tensor_scalar(out=lsm[:gw], in0=t1[:gw], scalar1=ln[:gw],
                                    scalar2=None, op0=ALU.subtract)
            nc.sync.dma_start(out=out_lg[gt * P:gt * P + gw, :], in_=lsm[:gw])

    nc.compile()
    return nc


# ------------------------------------------------------------------ entry

LAST_EXEC_NS = None

def kernel(x, edge_index, batch, W1, a_src1, a_dst1, b1, W2, a_src2, a_dst2, b2,
           fcW, fcb):
    x = np.asarray(x, np.float32)
    edge_index = np.asarray(edge_index, np.int64)
    batch = np.asarray(batch, np.int64)
    for b in (b1, b2, fcb):
        assert np.abs(np.asarray(b)).max() == 0.0, "nonzero bias unsupported"
    cfg = plan_cfg(N=x.shape[0], E0=edge_index.shape[1], G=512)
    in_maps = prep_inputs(cfg, x, edge_index, batch,
                          np.asarray(W1, np.float32), np.asarray(a_src1, np.float32),
                          np.asarray(a_dst1, np.float32), np.asarray(W2, np.float32),
                          np.asarray(a_src2, np.float32), np.asarray(a_dst2, np.float32),
                          np.asarray(fcW, np.float32))
    nc = build_nc(cfg)
    trace = os.environ.get("KERNEL_TRACE") == "1"
    res = run_bass_kernel_spmd(nc, in_maps, list(range(cfg.CORES)), trace=trace)
    global LAST_EXEC_NS
    LAST_EXEC_NS = res.exec_time_ns
    if trace:
        print(f"HW exec time: {res.exec_time_ns} ns "
              f"(mean {res.mean_exec_time_ns} ns)")
    return np.asarray(res.results[0]["out_lg"], np.float32)


# revision 18
# speedup vs baseline: 2.1021x; 1.0412x over previous
"""GAT classifier on 8 trn2 NeuronCores (Bass/Tile).

Sharding: 1D node partition (6250 nodes/core, padded to 6272 = 49 chunks of
128 slots); edges assigned to the core owning their dst node, grouped by dst
chunk. Host balances node->chunk assignment on per-half in-degree so the
padded per-(chunk,half) edge-section size is minimal. Self-loops are applied
locally in the finalize step (operands are chunk-local), not gathered.

Per 128-edge tile the segmented softmax + weighted aggregation is one
PSUM-accumulated "selection matmul": U[j, :] += Sh^T @ [u'h_rows | u'],
Sh[e, j] = S0[e, j] * wt[e, j], where S0[e, j] = (slot_e == j) is a STATIC
0/1 mask streamed from the host, and the weight uses the factorization
  exp(leaky_relu(as + ad)) / e^{0.2(as+ad)} = e^{0.8 relu(as + ad)} = wt
(the e^{0.2 as} factor is premultiplied into the source row as u' = e^{0.2 as},
and e^{0.2 ad} cancels in the softmax ratio). wt is built per tile by the
SCALAR engine from a per-chunk broadcast tile AD[e, j] = ad_j and a
per-partition bias as_e carried in the gathered row (fp32 inside the bf16
row), then masked with one DVE tensor_tensor. Node tables are bf16 (512B
rows L1, 256B rows L2), AllGathered between layers; pooling partials are
AllReduced.
"""
import math
import os
import sys
from contextlib import ExitStack
from dataclasses import dataclass

import numpy as np
import ml_dtypes

for _p in ("/opt/trn_rl_repo", "/root/.axon_site/_ro/trn_rl_repo"):
    if os.path.isdir(_p) and _p not in sys.path:
        sys.path.insert(0, _p)

import concourse.bacc as bacc
import concourse.bass as bass
import concourse.mybir as mybir
import concourse.tile as tile
from concourse.tile import add_dep_helper
from concourse.bass_utils import run_bass_kernel_spmd
from concourse.masks import make_identity

P = 128
AF = mybir.ActivationFunctionType
ALU = mybir.AluOpType
F32 = mybir.dt.float32
BF16 = mybir.dt.bfloat16
I16 = mybir.dt.int16
BF = ml_dtypes.bfloat16

ROW1 = 256  # bf16: [u'0*h0(64) u'0 u'1*h1(64) u'1 pad(2) f32:{as0 as1 as08_0 as08_1 u'0 u'1}]
ROW2 = 128  # bf16: [u'*h2(64) u' pad(1) f32:{as as08 u'}]
UO1 = 132   # bf16-elem offset of the fp32 block
UO2 = 66

# weight-build variant per (layer, head): "relu2" = 2 scalar acts + 1 DVE TT;
# "exp1" = 1 scalar act + 1 DVE TS + 1 DVE TT. Chosen to balance Scalar vs DVE.
VARIANT = {(1, 0): "exp1", (1, 1): "exp1", (2, 0): "exp1"}


@dataclass
class Cfg:
    N: int = 50000
    E0: int = 800000
    IN: int = 128
    HID: int = 64
    G: int = 512
    CORES: int = 8
    NPC: int = 0
    CH: int = 0
    SEC_LO: int = 0
    SEC_HI: int = 0
    T: int = 0
    T_LO: int = 0
    T_HI: int = 0
    G_CH: int = 2

    @property
    def NCH(self):  # padded per-core node count
        return self.CH * P

    @property
    def HALF(self):  # table rows of cores 0-3 (lo half for int16 indices)
        return (self.CORES // 2) * self.NCH

    @property
    def NTAB(self):
        return self.CORES * self.NCH


def plan_cfg(N, E0, G, CORES=8):
    c = Cfg(N=N, E0=E0, G=G, CORES=CORES)
    assert N % CORES == 0
    c.NPC = N // CORES
    c.CH = math.ceil(c.NPC / P)
    assert c.HALF <= 32767 and (c.NTAB - c.HALF) <= 32768
    return c


# ----------------------------------------------------------------- host prep

def balance_nodes(cfg, src, dst):
    """Assign nodes to (chunk, slot) per core, balancing per-half in-degree
    across chunks. Returns perm_rows: global node id -> global table row."""
    N, NPC, NCH, CH = cfg.N, cfg.NPC, cfg.NCH, cfg.CH
    deg = np.zeros((N, 2), np.int64)
    np.add.at(deg, (dst, ((src // NPC) >= (cfg.CORES // 2)).astype(np.int64)), 1)
    perm_rows = np.empty(N, np.int64)
    for c in range(cfg.CORES):
        dl = deg[c * NPC:(c + 1) * NPC].astype(np.float64)
        order = np.argsort(-(dl[:, 0] + dl[:, 1]), kind="stable")
        loads = np.zeros((CH, 2))
        cnts = np.zeros(CH, np.int64)
        for l in order:
            cand = np.maximum(loads[:, 0] + dl[l, 0], loads[:, 1] + dl[l, 1])
            cand[cnts >= P] = np.inf
            k = int(np.argmin(cand))
            perm_rows[c * NPC + l] = c * NCH + k * P + cnts[k]
            loads[k, 0] += dl[l, 0]
            loads[k, 1] += dl[l, 1]
            cnts[k] += 1
    return perm_rows


def prep_edges(cfg, src, dst, perm_rows):
    """Per-core edge index arrays + static S0 mask tiles. Fills cfg.SEC_*/T_*."""
    CH, NPC, NCH, HALF = cfg.CH, cfg.NPC, cfg.NCH, cfg.HALF
    per_core = []
    maxlo = maxhi = 0
    for c in range(cfg.CORES):
        m = (dst // NPC) == c
        srow = perm_rows[src[m]]
        dloc = perm_rows[dst[m]] - c * NCH
        chunk = dloc >> 7
        half = (srow >= HALF).astype(np.int64)
        order = np.lexsort((srow, half, chunk))
        srow, dloc, chunk, half = srow[order], dloc[order], chunk[order], half[order]
        key = chunk * 2 + half
        cnt = np.bincount(key, minlength=CH * 2).reshape(CH, 2)
        maxlo = max(maxlo, int(cnt[:, 0].max()))
        maxhi = max(maxhi, int(cnt[:, 1].max()))
        per_core.append((srow, dloc, cnt))
    cfg.SEC_LO = ((maxlo + 127) & ~127) or P
    cfg.SEC_HI = ((maxhi + 127) & ~127) or P
    cfg.T_LO = cfg.SEC_LO // P
    cfg.T_HI = cfg.SEC_HI // P
    cfg.T = cfg.T_LO + cfg.T_HI
    EC = cfg.SEC_LO + cfg.SEC_HI

    def wrap16(a):  # idx i -> [i % 16, i // 16], replicated over 8 groups
        w = a.reshape(-1, 16).T.copy()
        return np.tile(w, (8, 1)).astype(np.int16)

    jj = np.arange(P, dtype=np.int64)
    out = []
    for c in range(cfg.CORES):
        srow, dloc, cnt = per_core[c]
        gl = np.zeros((CH, cfg.SEC_LO), np.int16)
        gh = np.zeros((CH, cfg.SEC_HI), np.int16)
        sl = np.full((CH, EC), 512, np.int64)
        ofs = np.zeros(CH * 2 + 1, np.int64)
        np.cumsum(cnt.reshape(-1), out=ofs[1:])
        for k in range(CH):
            nlo, nhi = int(cnt[k, 0]), int(cnt[k, 1])
            a = ofs[2 * k]
            gl[k, :nlo] = srow[a:a + nlo]
            sl[k, :nlo] = dloc[a:a + nlo] & 127
            b = ofs[2 * k + 1]
            gh[k, :nhi] = srow[b:b + nhi] - cfg.HALF
            sl[k, cfg.SEC_LO:cfg.SEC_LO + nhi] = dloc[b:b + nhi] & 127
        # S0 mask tiles: [e, tile*128 + j] = (slot == j), tile-major
        s0 = (sl.reshape(CH * cfg.T, P)[:, :, None] == jj[None, None, :])
        s0 = np.ascontiguousarray(
            s0.transpose(1, 0, 2).reshape(P, CH * cfg.T * P)).astype(BF)
        out.append(dict(gl=wrap16(gl), gh=wrap16(gh), s0=s0))
    return out


def prep_inputs(cfg, x, edge_index, batch, W1, a_src1, a_dst1, W2, a_src2, a_dst2, fcW):
    N, CORES, NPC, NCH, CH = cfg.N, cfg.CORES, cfg.NPC, cfg.NCH, cfg.CH
    # self-loops are NOT in the gathered edge lists — they are applied
    # locally in the finalize step (all their operands are chunk-local)
    src = edge_index[0].astype(np.int64)
    dst = edge_index[1].astype(np.int64)
    perm_rows = balance_nodes(cfg, src, dst)
    edges = prep_edges(cfg, src, dst, perm_rows)

    H = 2
    HID = cfg.HID
    rhs1 = np.zeros((cfg.IN, 132), np.float32)
    rhs1[:, :H * HID] = W1
    for h in range(H):
        rhs1[:, H * HID + h] = W1[:, h * HID:(h + 1) * HID] @ a_src1[h]
        rhs1[:, H * HID + 2 + h] = W1[:, h * HID:(h + 1) * HID] @ a_dst1[h]
    rhs2 = np.zeros((H * HID, HID + 2), np.float32)
    rhs2[:, :HID] = W2
    rhs2[:, HID] = W2 @ a_src2[0]
    rhs2[:, HID + 1] = W2 @ a_dst2[0]

    iota512 = np.tile(np.arange(cfg.G, dtype=np.float32), (P, 1))
    cnt = np.bincount(batch, minlength=cfg.G).astype(np.float32)
    invc_b = np.tile(1.0 / np.maximum(cnt, 1.0), (HID, 1)).astype(np.float32)

    xT = np.zeros((cfg.IN, CORES * NCH), np.float32)
    gsl = np.full((CORES, NCH), 999.0, np.float32)
    for c in range(CORES):
        nodes = np.arange(c * NPC, (c + 1) * NPC)
        rows = perm_rows[nodes] - c * NCH
        xT[:, c * NCH + rows] = x[nodes].T
        gsl[c, rows] = batch[nodes]

    in_maps = []
    for c in range(CORES):
        in_maps.append(dict(
            xT=np.ascontiguousarray(xT[:, c * NCH:(c + 1) * NCH]),
            rhs1=rhs1, rhs2=rhs2, fcW=fcW.astype(np.float32),
            iota512=iota512, invc=invc_b,
            gslot=gsl[c].reshape(CH, P).T.copy(),
            **edges[c],
        ))
    return in_maps


# -------------------------------------------------------------- bass builder

def build_nc(cfg):
    CH, T, T_LO, T_HI = cfg.CH, cfg.T, cfg.T_LO, cfg.T_HI
    SEC_LO, SEC_HI = cfg.SEC_LO, cfg.SEC_HI
    HID, G, NCH, HALF, NTAB = cfg.HID, cfg.G, cfg.NCH, cfg.HALF, cfg.NTAB
    R = list(range(cfg.CORES))

    nc = bacc.Bacc()
    pi = lambda n, s, d=F32: nc.declare_dram_parameter(n, s, d, isOutput=False)
    xT = pi("xT", [cfg.IN, NCH])
    rhs1 = pi("rhs1", [cfg.IN, 132])
    rhs2 = pi("rhs2", [2 * HID, HID + 2])
    fcW = pi("fcW", [HID, 2])
    iota512 = pi("iota512", [P, G])
    invc = pi("invc", [HID, G])
    gslot = pi("gslot", [P, CH])
    gl = pi("gl", [P, CH * SEC_LO // 16], I16)
    gh = pi("gh", [P, CH * SEC_HI // 16], I16)
    s0p = pi("s0", [P, CH * T * P], BF16)
    out_lg = nc.declare_dram_parameter("out_lg", [G, 2], F32, isOutput=True)

    shard1 = nc.dram_tensor("shard1", [NCH, ROW1], BF16)
    table1 = nc.dram_tensor("table1", [NTAB, ROW1], BF16, addr_space="Shared")
    shard2 = nc.dram_tensor("shard2", [NCH, ROW2], BF16)
    table2 = nc.dram_tensor("table2", [NTAB, ROW2], BF16, addr_space="Shared")
    pool_loc = nc.dram_tensor("pool_loc", [HID, G], F32)
    pool_sh = nc.dram_tensor("pool_sh", [HID, G], F32, addr_space="Shared")

    groups = [tuple(range(a, min(a + cfg.G_CH, CH))) for a in range(0, CH, cfg.G_CH)]

    # SWDGE descriptor-ring pacing (see baseline): a probe marks gather
    # completion; later gathers dep on the probe to bound outstanding entries.
    # The probe is a tiny SYNC-engine DMA (not a DVE op): the sync queue is
    # nearly empty, so the probe fires as soon as the gather's DMA lands
    # instead of queueing behind a group's worth of DVE tile ops.
    gather_fifo = []

    def paced_gather(probe_pool, **kw):
        e = kw["num_idxs"] // 16 + 1
        inst = nc.gpsimd.dma_gather(single_packet=False, **kw)
        gp_t = probe_pool.tile([1, 2], BF16, tag="gprobe", name="gprobe")
        rd = nc.sync.dma_start(out=gp_t[:], in_=kw["out_ap"][0:1, 0, 0:2])
        tot = sum(x[1] for x in gather_fifo) + e
        while gather_fifo and (tot > 110 or len(gather_fifo) >= 2):
            _, eo, rdo = gather_fifo.pop(0)
            add_dep_helper(inst.ins, rdo.ins, sync=True, reason="swdge ring pacing")
            tot -= eo
        gather_fifo.append((inst, e, rd))
        return inst

    with tile.TileContext(nc) as tc, ExitStack() as ctx:
        cp = ctx.enter_context(tc.tile_pool(name="const", bufs=1))
        dio = ctx.enter_context(tc.tile_pool(name="dio", bufs=3))
        dps = ctx.enter_context(tc.tile_pool(name="dps", bufs=2, space="PSUM"))
        o1p = ctx.enter_context(tc.tile_pool(name="o1p", bufs=1))
        ixp = ctx.enter_context(tc.tile_pool(name="ixp", bufs=2))
        s0pl = ctx.enter_context(tc.tile_pool(name="s0pl", bufs=3))
        gp = ctx.enter_context(tc.tile_pool(name="gp", bufs=2))
        sxp = ctx.enter_context(tc.tile_pool(name="sxp", bufs=6))
        xp = ctx.enter_context(tc.tile_pool(name="xp", bufs=3))
        ups = ctx.enter_context(tc.tile_pool(name="ups", bufs=2, space="PSUM"))
        pps = ctx.enter_context(tc.tile_pool(name="pps", bufs=1, space="PSUM"))
        fin = ctx.enter_context(tc.tile_pool(name="fin", bufs=3))

        io512 = cp.tile([P, G], F32)
        nc.sync.dma_start(out=io512[:], in_=iota512[:])
        r1sb = cp.tile([cfg.IN, 132], F32)
        nc.sync.dma_start(out=r1sb[:], in_=rhs1[:])
        r2sb = cp.tile([2 * HID, HID + 2], F32)
        nc.sync.dma_start(out=r2sb[:], in_=rhs2[:])
        fcsb = cp.tile([HID, 2], F32)
        nc.sync.dma_start(out=fcsb[:], in_=fcW[:])
        icsb = cp.tile([HID, G], F32)
        nc.sync.dma_start(out=icsb[:], in_=invc[:])
        gssb = cp.tile([P, CH], F32)
        nc.sync.dma_start(out=gssb[:], in_=gslot[:])
        idsb = cp.tile([P, P], F32)
        make_identity(nc, idsb[:])
        onesW = cp.tile([P, P], F32)
        nc.vector.memset(onesW[:], 1.0)
        out1 = o1p.tile([P, CH * P], F32)
        AD1 = cp.tile([P, CH * 2 * P], BF16)
        AD2 = cp.tile([P, CH * P], BF16)
        adcol1 = cp.tile([P, 2 * CH], F32)
        adcol2 = cp.tile([P, CH], F32)
        rows1 = cp.tile([P, CH * ROW1], BF16)  # local node rows (self-loop term)
        rows2 = cp.tile([P, CH * ROW2], BF16)
        nc.vector.memset(rows1[:], 0.0)
        nc.vector.memset(rows2[:], 0.0)

        # ---------------- dense 1: rows of table1 + ad columns ----------------
        for t in range(CH):
            xt = dio.tile([P, P], F32, tag="xt")
            nc.sync.dma_start(out=xt[:], in_=xT[:, t * P:(t + 1) * P])
            ps = dps.tile([P, 132], F32, tag="dtmp")
            nc.tensor.matmul(out=ps[:], lhsT=xt[:], rhs=r1sb[:], start=True, stop=True)
            upc = fin.tile([P, 2], F32, tag="upc")
            nc.scalar.activation(out=upc[:], in_=ps[:, 128:130], func=AF.Exp, scale=0.2)
            row = rows1[:, t * ROW1:(t + 1) * ROW1]
            nc.scalar.activation(out=row[:, 0:64], in_=ps[:, 0:64], func=AF.Copy,
                                 scale=upc[:, 0:1])
            nc.scalar.activation(out=row[:, 65:129], in_=ps[:, 64:128], func=AF.Copy,
                                 scale=upc[:, 1:2])
            nc.vector.tensor_copy(out=row[:, 64:65], in_=upc[:, 0:1])
            nc.vector.tensor_copy(out=row[:, 129:130], in_=upc[:, 1:2])
            um = row[:, UO1:UO1 + 12].bitcast(F32)
            nc.vector.tensor_copy(out=um[:, 0:2], in_=ps[:, 128:130])
            nc.vector.tensor_scalar(out=um[:, 2:4], in0=ps[:, 128:130],
                                    scalar1=0.8, scalar2=None, op0=ALU.mult)
            nc.vector.tensor_copy(out=um[:, 4:6], in_=upc[:])
            nc.vector.tensor_copy(out=adcol1[:, 2 * t:2 * t + 2], in_=ps[:, 130:132])
            nc.sync.dma_start(out=shard1[t * P:(t + 1) * P, :], in_=row[:])

        # AD: per (chunk, head) broadcast of ad over the free dim — replicate
        # the ad column along free (tensor_scalar vs ones), PE-transpose, copy.
        def build_ad(adcol_t, ncols, ad_t):
            for i in range(ncols):
                rc = fin.tile([P, P], F32, tag="rc")
                nc.vector.tensor_scalar(out=rc[:], in0=onesW[:],
                                        scalar1=adcol_t[:, i:i + 1], scalar2=None,
                                        op0=ALU.mult)
                vps = dps.tile([P, P], F32, tag="dtmp")
                nc.tensor.transpose(out=vps[:], in_=rc[:], identity=idsb[:])
                nc.scalar.copy(out=ad_t[:, i * P:(i + 1) * P], in_=vps[:])

        tc.strict_bb_all_engine_barrier()
        nc.gpsimd.collective_compute(
            "AllGather", ALU.bypass, replica_groups=[R],
            ins=[shard1[:]], outs=[table1[:]])
        build_ad(adcol1, 2 * CH, AD1)  # overlaps the AllGather

        # ---------------- edge phase (shared for both layers) ----------------
        def edge_layer(layer, tabA, tabB, ad_t, row_w, uo, nheads, finalize):
            for grp in groups:
                g0, ng = grp[0], len(grp)
                nlo, nhi = ng * SEC_LO, ng * SEC_HI
                glt = ixp.tile([P, nlo // 16], I16, tag="glt")
                nc.sync.dma_start(out=glt[:], in_=gl[:, g0 * SEC_LO // 16:(g0 * SEC_LO + nlo) // 16])
                ght = ixp.tile([P, nhi // 16], I16, tag="ght")
                nc.sync.dma_start(out=ght[:], in_=gh[:, g0 * SEC_HI // 16:(g0 * SEC_HI + nhi) // 16])
                s0t = s0pl.tile([P, ng * T * P], BF16, tag="s0t")
                nc.sync.dma_start(out=s0t[:], in_=s0p[:, g0 * T * P:(g0 + ng) * T * P])
                hgl = gp.tile([P, nlo // P, row_w], BF16, tag="hgl")
                paced_gather(xp, out_ap=hgl[:], in_ap=tabA, idxs_ap=glt[:],
                             num_idxs=nlo, num_idxs_reg=nlo, elem_size=row_w)
                hgh = gp.tile([P, nhi // P, row_w], BF16, tag="hgh")
                paced_gather(xp, out_ap=hgh[:], in_ap=tabB, idxs_ap=ght[:],
                             num_idxs=nhi, num_idxs_reg=nhi, elem_size=row_w)
                for ci, c in enumerate(grp):
                    Us = [ups.tile([P, 65], F32, tag=f"U{h}", name=f"U{h}")
                          for h in range(nheads)]
                    for t in range(T):
                        if t < T_LO:
                            hg_t, tt, nt = hgl, t, T_LO
                        else:
                            hg_t, tt, nt = hgh, t - T_LO, T_HI
                        base = ci * nt + tt
                        s0_t = s0t[:, ((ci * T) + t) * P:((ci * T) + t + 1) * P]
                        uf = hg_t[:, base, uo:uo + 6 * nheads].bitcast(F32)
                        for h in range(nheads):
                            adsl = ad_t[:, (c * nheads + h) * P:(c * nheads + h + 1) * P]
                            Sh = sxp.tile([P, P], BF16, tag=f"Sh{h}")
                            if VARIANT[(layer, h)] == "relu2":
                                Y = sxp.tile([P, P], BF16, tag=f"Y{h}")
                                nc.scalar.activation(out=Y[:], in_=adsl, func=AF.Relu,
                                                     scale=1.0, bias=uf[:, h:h + 1])
                                Gt = sxp.tile([P, P], BF16, tag=f"G{h}")
                                nc.scalar.activation(out=Gt[:], in_=Y[:], func=AF.Exp,
                                                     scale=0.8)
                            else:  # exp1
                                E = sxp.tile([P, P], BF16, tag=f"E{h}")
                                nc.scalar.activation(out=E[:], in_=adsl, func=AF.Exp,
                                                     scale=0.8,
                                                     bias=uf[:, nheads + h:nheads + h + 1])
                                Gt = sxp.tile([P, P], BF16, tag=f"G{h}")
                                nc.vector.tensor_scalar(out=Gt[:], in0=E[:],
                                                        scalar1=1.0, scalar2=None,
                                                        op0=ALU.max)
                            nc.vector.tensor_tensor(out=Sh[:], in0=s0_t, in1=Gt[:],
                                                    op=ALU.mult)
                            nc.tensor.matmul(
                                out=Us[h][:], lhsT=Sh[:],
                                rhs=hg_t[:, base, h * 65:(h + 1) * 65],
                                start=(t == 0), stop=(t == T - 1))
                    finalize(c, Us)

        # self-loop term: wt = e^{0.8 relu(as+ad)}, num += wt*(u'h), den += wt*u'
        def self_w(c, h, rows_t, adcol_t, row_w, uo, nheads):
            uf = rows_t[:, c * row_w + uo:c * row_w + uo + 6 * nheads].bitcast(F32)
            E = fin.tile([P, 1], F32, tag="Ew")
            nc.scalar.activation(out=E[:], in_=adcol_t[:, nheads * c + h:nheads * c + h + 1],
                                 func=AF.Exp, scale=0.8,
                                 bias=uf[:, nheads + h:nheads + h + 1])
            W = fin.tile([P, 1], F32, tag="Ww")
            nc.vector.tensor_scalar(out=W[:], in0=E[:], scalar1=1.0, scalar2=None,
                                    op0=ALU.max)
            wst = fin.tile([P, 1], F32, tag="wst")
            nc.vector.tensor_tensor(out=wst[:], in0=W[:],
                                    in1=uf[:, 2 * nheads + h:2 * nheads + h + 1],
                                    op=ALU.mult)
            numv = fin.tile([P, 64], F32, tag="numv")
            nc.vector.tensor_scalar(out=numv[:], in0=rows_t[:, c * row_w + h * 65:c * row_w + h * 65 + 64],
                                    scalar1=W[:], scalar2=None, op0=ALU.mult)
            return wst, numv

        def dense2_chunk(t):
            tp = dps.tile([P, P], F32, tag="dtmp")
            nc.tensor.transpose(out=tp[:], in_=out1[:, t * P:(t + 1) * P], identity=idsb[:])
            h1T = dio.tile([P, P], F32, tag="h1T")
            nc.scalar.copy(out=h1T[:], in_=tp[:])
            ps = dps.tile([P, HID + 2], F32, tag="dtmp")
            nc.tensor.matmul(out=ps[:], lhsT=h1T[:], rhs=r2sb[:], start=True, stop=True)
            upc = fin.tile([P, 1], F32, tag="upc2")
            nc.scalar.activation(out=upc[:], in_=ps[:, 64:65], func=AF.Exp, scale=0.2)
            row = rows2[:, t * ROW2:(t + 1) * ROW2]
            nc.scalar.activation(out=row[:, 0:64], in_=ps[:, 0:64], func=AF.Copy,
                                 scale=upc[:, 0:1])
            nc.vector.tensor_copy(out=row[:, 64:65], in_=upc[:, 0:1])
            um = row[:, UO2:UO2 + 6].bitcast(F32)
            nc.vector.tensor_copy(out=um[:, 0:1], in_=ps[:, 64:65])
            nc.vector.tensor_scalar(out=um[:, 1:2], in0=ps[:, 64:65],
                                    scalar1=0.8, scalar2=None, op0=ALU.mult)
            nc.vector.tensor_copy(out=um[:, 2:3], in_=upc[:])
            nc.vector.tensor_copy(out=adcol2[:, t:t + 1], in_=ps[:, 65:66])
            nc.sync.dma_start(out=shard2[t * P:(t + 1) * P, :], in_=row[:])

        def fin1(c, Us):
            for h in range(2):
                wst, numv = self_w(c, h, rows1, adcol1, ROW1, UO1, 2)
                un = fin.tile([P, 64], F32, tag="un1")
                nc.vector.tensor_tensor(out=un[:], in0=Us[h][:, 0:64], in1=numv[:],
                                        op=ALU.add)
                den = fin.tile([P, 1], F32, tag="den1")
                nc.vector.tensor_tensor(out=den[:], in0=Us[h][:, 64:65], in1=wst[:],
                                        op=ALU.add)
                rd = fin.tile([P, 1], F32, tag="rd1")
                nc.vector.reciprocal(out=rd[:], in_=den[:])
                nc.vector.tensor_scalar(
                    out=out1[:, c * P + h * 64:c * P + (h + 1) * 64],
                    in0=un[:], scalar1=rd[:], scalar2=0.0,
                    op0=ALU.mult, op1=ALU.max)
            dense2_chunk(c)  # layer-2 dense work rides along under edge-1

        edge_layer(1, table1[0:HALF, :], table1[HALF:NTAB, :], AD1, ROW1, UO1, 2, fin1)

        tc.strict_bb_all_engine_barrier()
        nc.gpsimd.collective_compute(
            "AllGather", ALU.bypass, replica_groups=[R],
            ins=[shard2[:]], outs=[table2[:]])
        build_ad(adcol2, CH, AD2)  # overlaps the AllGather

        # ---------------- edge layer 2 + pooling ----------------
        plT = pps.tile([HID, G], F32)

        def fin2(c, Us):
            wst, numv = self_w(c, 0, rows2, adcol2, ROW2, UO2, 1)
            un = fin.tile([P, 64], F32, tag="un2")
            nc.vector.tensor_tensor(out=un[:], in0=Us[0][:, 0:64], in1=numv[:],
                                    op=ALU.add)
            den = fin.tile([P, 1], F32, tag="den2")
            nc.vector.tensor_tensor(out=den[:], in0=Us[0][:, 64:65], in1=wst[:],
                                    op=ALU.add)
            rd = fin.tile([P, 1], F32, tag="rd2")
            nc.vector.reciprocal(out=rd[:], in_=den[:])
            o2 = fin.tile([P, HID], F32, tag="o2")
            nc.vector.tensor_scalar(out=o2[:], in0=un[:],
                                    scalar1=rd[:], scalar2=0.0,
                                    op0=ALU.mult, op1=ALU.max)
            sg = fin.tile([P, G], F32, tag="sg")
            nc.vector.tensor_scalar(out=sg[:], in0=io512[:],
                                    scalar1=gssb[:, c:c + 1], scalar2=None,
                                    op0=ALU.is_equal)
            nc.tensor.matmul(out=plT[:], lhsT=o2[:], rhs=sg[:],
                             start=(c == 0), stop=(c == CH - 1))

        edge_layer(2, table2[0:HALF, :], table2[HALF:NTAB, :], AD2, ROW2, UO2, 1, fin2)

        plsb = fin.tile([HID, G], F32)
        nc.vector.tensor_copy(out=plsb[:], in_=plT[:])
        nc.sync.dma_start(out=pool_loc[:], in_=plsb[:])
        tc.strict_bb_all_engine_barrier()
        nc.gpsimd.collective_compute(
            "AllReduce", ALU.add, replica_groups=[R],
            ins=[pool_loc[:]], outs=[pool_sh[:]])
        plr = fin.tile([HID, G], F32)
        nc.sync.dma_start(out=plr[:], in_=pool_sh[:])
        nc.vector.tensor_tensor(out=plr[:], in0=plr[:], in1=icsb[:], op=ALU.mult)
        for gt in range(max(1, G // P)):
            gw = min(P, G - gt * P)
            lg = dps.tile([P, 2], F32, tag="dtmp")
            nc.tensor.matmul(out=lg[:gw], lhsT=plr[:, gt * P:gt * P + gw], rhs=fcsb[:],
                             start=True, stop=True)
            mx = fin.tile([P, 1], F32, tag="mx")
            nc.vector.tensor_reduce(out=mx[:gw], in_=lg[:gw], op=ALU.max,
                                    axis=mybir.AxisListType.X)
            t1 = fin.tile([P, 2], F32, tag="t1")
            nc.vector.tensor_scalar(out=t1[:gw], in0=lg[:gw], scalar1=mx[:gw],
                                    scalar2=None, op0=ALU.subtract)
            ex = fin.tile([P, 2], F32, tag="ex")
            es = fin.tile([P, 1], F32, tag="es")
            nc.scalar.activation(out=ex[:gw], in_=t1[:gw], func=AF.Exp, accum_out=es[:gw])
            ln = fin.tile([P, 1], F32, tag="ln")
            nc.scalar.activation(out=ln[:gw], in_=es[:gw], func=AF.Ln)
            lsm = fin.tile([P, 2], F32, tag="lsm")
            nc.vector.tensor_scalar(out=lsm[:gw], in0=t1[:gw], scalar1=ln[:gw],
                                    scalar2=None, op0=ALU.subtract)
            nc.sync.dma_start(out=out_lg[gt * P:gt * P + gw, :], in_=lsm[:gw])

    nc.compile()
    return nc


# ------------------------------------------------------------------ entry

LAST_EXEC_NS = None

def kernel(x, edge_index, batch, W1, a_src1, a_dst1, b1, W2, a_src2, a_dst2, b2,
           fcW, fcb):
    x = np.asarray(x, np.float32)
    edge_index = np.asarray(edge_index, np.int64)
    batch = np.asarray(batch, np.int64)
    for b in (b1, b2, fcb):
        assert np.abs(np.asarray(b)).max() == 0.0, "nonzero bias unsupported"
    cfg = plan_cfg(N=x.shape[0], E0=edge_index.shape[1], G=512)
    in_maps = prep_inputs(cfg, x, edge_index, batch,
                          np.asarray(W1, np.float32), np.asarray(a_src1, np.float32),
                          np.asarray(a_dst1, np.float32), np.asarray(W2, np.float32),
                          np.asarray(a_src2, np.float32), np.asarray(a_dst2, np.float32),
                          np.asarray(fcW, np.float32))
    nc = build_nc(cfg)
    trace = os.environ.get("KERNEL_TRACE") == "1"
    res = run_bass_kernel_spmd(nc, in_maps, list(range(cfg.CORES)), trace=trace)
    global LAST_EXEC_NS
    LAST_EXEC_NS = res.exec_time_ns
    if trace:
        print(f"HW exec time: {res.exec_time_ns} ns "
              f"(mean {res.mean_exec_time_ns} ns)")
    return np.asarray(res.results[0]["out_lg"], np.float32)


# revision 21
# speedup vs baseline: 2.1286x; 1.0126x over previous
"""GAT classifier on 8 trn2 NeuronCores (Bass/Tile).

Sharding: 1D node partition (6250 nodes/core, padded to 6272 = 49 chunks of
128 slots); edges assigned to the core owning their dst node, grouped by dst
chunk. Host balances node->chunk assignment on per-half in-degree so the
padded per-(chunk,half) edge-section size is minimal. Self-loops are applied
locally in the finalize step (operands are chunk-local), not gathered.

Per 128-edge tile the segmented softmax + weighted aggregation is one
PSUM-accumulated "selection matmul": U[j, :] += Sh^T @ [u'h_rows | u'],
Sh[e, j] = S0[e, j] * wt[e, j], where S0[e, j] = (slot_e == j) is a STATIC
0/1 mask streamed from the host, and the weight uses the factorization
  exp(leaky_relu(as + ad)) / e^{0.2(as+ad)} = e^{0.8 relu(as + ad)} = wt
(the e^{0.2 as} factor is premultiplied into the source row as u' = e^{0.2 as},
and e^{0.2 ad} cancels in the softmax ratio). wt is built per tile by the
SCALAR engine from a per-chunk broadcast tile AD[e, j] = ad_j and a
per-partition bias as_e carried in the gathered row (fp32 inside the bf16
row), then masked with one DVE tensor_tensor. Node tables are bf16 (512B
rows L1, 256B rows L2), AllGathered between layers; pooling partials are
AllReduced.
"""
import math
import os
import sys
from contextlib import ExitStack
from dataclasses import dataclass

import numpy as np
import ml_dtypes

for _p in ("/opt/trn_rl_repo", "/root/.axon_site/_ro/trn_rl_repo"):
    if os.path.isdir(_p) and _p not in sys.path:
        sys.path.insert(0, _p)

import concourse.bacc as bacc
import concourse.bass as bass
import concourse.mybir as mybir
import concourse.tile as tile
from concourse.tile import add_dep_helper
from concourse.bass_utils import run_bass_kernel_spmd
from concourse.masks import make_identity

P = 128
AF = mybir.ActivationFunctionType
ALU = mybir.AluOpType
F32 = mybir.dt.float32
BF16 = mybir.dt.bfloat16
I16 = mybir.dt.int16
BF = ml_dtypes.bfloat16

ROW1 = 256  # bf16: [u'0*h0(64) u'0 u'1*h1(64) u'1 pad(2) f32:{as0 as1 as08_0 as08_1 u'0 u'1}]
ROW2 = 128  # bf16: [u'*h2(64) u' pad(1) f32:{as as08 u'}]
UO1 = 132   # bf16-elem offset of the fp32 block
UO2 = 66

# weight-build variant per (layer, head): "relu2" = 2 scalar acts + 1 DVE TT;
# "exp1" = 1 scalar act + 1 DVE TS + 1 DVE TT. Chosen to balance Scalar vs DVE.
VARIANT = {(1, 0): "exp1", (1, 1): "exp1", (2, 0): "exp1"}


@dataclass
class Cfg:
    N: int = 50000
    E0: int = 800000
    IN: int = 128
    HID: int = 64
    G: int = 512
    CORES: int = 8
    NPC: int = 0
    CH: int = 0
    SEC_LO: int = 0
    SEC_HI: int = 0
    T: int = 0
    T_LO: int = 0
    T_HI: int = 0
    G_CH: int = 2

    @property
    def NCH(self):  # padded per-core node count
        return self.CH * P

    @property
    def HALF(self):  # table rows of cores 0-3 (lo half for int16 indices)
        return (self.CORES // 2) * self.NCH

    @property
    def NTAB(self):
        return self.CORES * self.NCH


def plan_cfg(N, E0, G, CORES=8):
    c = Cfg(N=N, E0=E0, G=G, CORES=CORES)
    assert N % CORES == 0
    c.NPC = N // CORES
    c.CH = math.ceil(c.NPC / P)
    assert c.HALF <= 32767 and (c.NTAB - c.HALF) <= 32768
    return c


# ----------------------------------------------------------------- host prep

def balance_nodes(cfg, src, dst):
    """Assign nodes to (chunk, slot) per core, balancing per-half in-degree
    across chunks. Returns perm_rows: global node id -> global table row."""
    N, NPC, NCH, CH = cfg.N, cfg.NPC, cfg.NCH, cfg.CH
    deg = np.zeros((N, 2), np.int64)
    np.add.at(deg, (dst, ((src // NPC) >= (cfg.CORES // 2)).astype(np.int64)), 1)
    perm_rows = np.empty(N, np.int64)
    for c in range(cfg.CORES):
        dl = deg[c * NPC:(c + 1) * NPC].astype(np.float64)
        order = np.argsort(-(dl[:, 0] + dl[:, 1]), kind="stable")
        loads = np.zeros((CH, 2))
        cnts = np.zeros(CH, np.int64)
        for l in order:
            cand = np.maximum(loads[:, 0] + dl[l, 0], loads[:, 1] + dl[l, 1])
            cand[cnts >= P] = np.inf
            k = int(np.argmin(cand))
            perm_rows[c * NPC + l] = c * NCH + k * P + cnts[k]
            loads[k, 0] += dl[l, 0]
            loads[k, 1] += dl[l, 1]
            cnts[k] += 1
    return perm_rows


def prep_edges(cfg, src, dst, perm_rows):
    """Per-core edge index arrays + static S0 mask tiles. Fills cfg.SEC_*/T_*."""
    CH, NPC, NCH, HALF = cfg.CH, cfg.NPC, cfg.NCH, cfg.HALF
    per_core = []
    maxlo = maxhi = 0
    for c in range(cfg.CORES):
        m = (dst // NPC) == c
        srow = perm_rows[src[m]]
        dloc = perm_rows[dst[m]] - c * NCH
        chunk = dloc >> 7
        half = (srow >= HALF).astype(np.int64)
        order = np.lexsort((srow, half, chunk))
        srow, dloc, chunk, half = srow[order], dloc[order], chunk[order], half[order]
        key = chunk * 2 + half
        cnt = np.bincount(key, minlength=CH * 2).reshape(CH, 2)
        maxlo = max(maxlo, int(cnt[:, 0].max()))
        maxhi = max(maxhi, int(cnt[:, 1].max()))
        per_core.append((srow, dloc, cnt))
    cfg.SEC_LO = ((maxlo + 127) & ~127) or P
    cfg.SEC_HI = ((maxhi + 127) & ~127) or P
    cfg.T_LO = cfg.SEC_LO // P
    cfg.T_HI = cfg.SEC_HI // P
    cfg.T = cfg.T_LO + cfg.T_HI
    EC = cfg.SEC_LO + cfg.SEC_HI

    def wrap16(a):  # idx i -> [i % 16, i // 16], replicated over 8 groups
        w = a.reshape(-1, 16).T.copy()
        return np.tile(w, (8, 1)).astype(np.int16)

    jj = np.arange(P, dtype=np.int64)
    out = []
    for c in range(cfg.CORES):
        srow, dloc, cnt = per_core[c]
        gl = np.zeros((CH, cfg.SEC_LO), np.int16)
        gh = np.zeros((CH, cfg.SEC_HI), np.int16)
        sl = np.full((CH, EC), 512, np.int64)
        ofs = np.zeros(CH * 2 + 1, np.int64)
        np.cumsum(cnt.reshape(-1), out=ofs[1:])
        for k in range(CH):
            nlo, nhi = int(cnt[k, 0]), int(cnt[k, 1])
            a = ofs[2 * k]
            gl[k, :nlo] = srow[a:a + nlo]
            sl[k, :nlo] = dloc[a:a + nlo] & 127
            b = ofs[2 * k + 1]
            gh[k, :nhi] = srow[b:b + nhi] - cfg.HALF
            sl[k, cfg.SEC_LO:cfg.SEC_LO + nhi] = dloc[b:b + nhi] & 127
        # S0 mask tiles: [e, tile*128 + j] = (slot == j), tile-major
        s0 = (sl.reshape(CH * cfg.T, P)[:, :, None] == jj[None, None, :])
        s0 = np.ascontiguousarray(
            s0.transpose(1, 0, 2).reshape(P, CH * cfg.T * P)).astype(BF)
        out.append(dict(gl=wrap16(gl), gh=wrap16(gh), s0=s0))
    return out


def prep_inputs(cfg, x, edge_index, batch, W1, a_src1, a_dst1, W2, a_src2, a_dst2, fcW):
    N, CORES, NPC, NCH, CH = cfg.N, cfg.CORES, cfg.NPC, cfg.NCH, cfg.CH
    # self-loops are NOT in the gathered edge lists — they are applied
    # locally in the finalize step (all their operands are chunk-local)
    src = edge_index[0].astype(np.int64)
    dst = edge_index[1].astype(np.int64)
    perm_rows = balance_nodes(cfg, src, dst)
    edges = prep_edges(cfg, src, dst, perm_rows)

    H = 2
    HID = cfg.HID
    rhs1 = np.zeros((cfg.IN, 132), np.float32)
    rhs1[:, :H * HID] = W1
    for h in range(H):
        rhs1[:, H * HID + h] = W1[:, h * HID:(h + 1) * HID] @ a_src1[h]
        rhs1[:, H * HID + 2 + h] = W1[:, h * HID:(h + 1) * HID] @ a_dst1[h]
    rhs2 = np.zeros((H * HID, HID + 2), np.float32)
    rhs2[:, :HID] = W2
    rhs2[:, HID] = W2 @ a_src2[0]
    rhs2[:, HID + 1] = W2 @ a_dst2[0]

    iota512 = np.tile(np.arange(cfg.G, dtype=np.float32), (P, 1))
    cnt = np.bincount(batch, minlength=cfg.G).astype(np.float32)
    invc_b = np.tile(1.0 / np.maximum(cnt, 1.0), (HID, 1)).astype(np.float32)

    xT = np.zeros((cfg.IN, CORES * NCH), np.float32)
    gsl = np.full((CORES, NCH), 999.0, np.float32)
    for c in range(CORES):
        nodes = np.arange(c * NPC, (c + 1) * NPC)
        rows = perm_rows[nodes] - c * NCH
        xT[:, c * NCH + rows] = x[nodes].T
        gsl[c, rows] = batch[nodes]

    in_maps = []
    for c in range(CORES):
        in_maps.append(dict(
            xT=np.ascontiguousarray(xT[:, c * NCH:(c + 1) * NCH]),
            rhs1=rhs1, rhs2=rhs2, fcW=fcW.astype(np.float32),
            iota512=iota512, invc=invc_b,
            gslot=gsl[c].reshape(CH, P).T.copy(),
            **edges[c],
        ))
    return in_maps


# -------------------------------------------------------------- bass builder

def build_nc(cfg):
    CH, T, T_LO, T_HI = cfg.CH, cfg.T, cfg.T_LO, cfg.T_HI
    SEC_LO, SEC_HI = cfg.SEC_LO, cfg.SEC_HI
    HID, G, NCH, HALF, NTAB = cfg.HID, cfg.G, cfg.NCH, cfg.HALF, cfg.NTAB
    R = list(range(cfg.CORES))

    nc = bacc.Bacc()
    pi = lambda n, s, d=F32: nc.declare_dram_parameter(n, s, d, isOutput=False)
    xT = pi("xT", [cfg.IN, NCH])
    rhs1 = pi("rhs1", [cfg.IN, 132])
    rhs2 = pi("rhs2", [2 * HID, HID + 2])
    fcW = pi("fcW", [HID, 2])
    iota512 = pi("iota512", [P, G])
    invc = pi("invc", [HID, G])
    gslot = pi("gslot", [P, CH])
    gl = pi("gl", [P, CH * SEC_LO // 16], I16)
    gh = pi("gh", [P, CH * SEC_HI // 16], I16)
    s0p = pi("s0", [P, CH * T * P], BF16)
    out_lg = nc.declare_dram_parameter("out_lg", [G, 2], F32, isOutput=True)

    shard1 = nc.dram_tensor("shard1", [NCH, ROW1], BF16)
    table1 = nc.dram_tensor("table1", [NTAB, ROW1], BF16, addr_space="Shared")
    shard2 = nc.dram_tensor("shard2", [NCH, ROW2], BF16)
    table2 = nc.dram_tensor("table2", [NTAB, ROW2], BF16, addr_space="Shared")
    pool_loc = nc.dram_tensor("pool_loc", [HID, G], F32)
    pool_sh = nc.dram_tensor("pool_sh", [HID, G], F32, addr_space="Shared")

    groups = [tuple(range(a, min(a + cfg.G_CH, CH))) for a in range(0, CH, cfg.G_CH)]

    # SWDGE descriptor-ring pacing (see baseline): a probe marks gather
    # completion; later gathers dep on the probe to bound outstanding entries.
    # The probe is a tiny SYNC-engine DMA (not a DVE op): the sync queue is
    # nearly empty, so the probe fires as soon as the gather's DMA lands
    # instead of queueing behind a group's worth of DVE tile ops.
    gather_fifo = []

    def paced_gather(probe_pool, **kw):
        e = kw["num_idxs"] // 16 + 1
        inst = nc.gpsimd.dma_gather(single_packet=False, **kw)
        gp_t = probe_pool.tile([1, 2], BF16, tag="gprobe", name="gprobe")
        rd = nc.sync.dma_start(out=gp_t[:], in_=kw["out_ap"][0:1, 0, 0:2])
        tot = sum(x[1] for x in gather_fifo) + e
        while gather_fifo and (tot > 110 or len(gather_fifo) >= 2):
            _, eo, rdo = gather_fifo.pop(0)
            add_dep_helper(inst.ins, rdo.ins, sync=True, reason="swdge ring pacing")
            tot -= eo
        gather_fifo.append((inst, e, rd))
        return inst

    with tile.TileContext(nc) as tc, ExitStack() as ctx:
        cp = ctx.enter_context(tc.tile_pool(name="const", bufs=1))
        dio = ctx.enter_context(tc.tile_pool(name="dio", bufs=3))
        dps = ctx.enter_context(tc.tile_pool(name="dps", bufs=2, space="PSUM"))
        o1p = ctx.enter_context(tc.tile_pool(name="o1p", bufs=1))
        ixp = ctx.enter_context(tc.tile_pool(name="ixp", bufs=2))
        s0pl = ctx.enter_context(tc.tile_pool(name="s0pl", bufs=2))
        gp = ctx.enter_context(tc.tile_pool(name="gp", bufs=2))
        sxp = ctx.enter_context(tc.tile_pool(name="sxp", bufs=4))
        xp = ctx.enter_context(tc.tile_pool(name="xp", bufs=3))
        ups = ctx.enter_context(tc.tile_pool(name="ups", bufs=2, space="PSUM"))
        pps = ctx.enter_context(tc.tile_pool(name="pps", bufs=1, space="PSUM"))
        fin = ctx.enter_context(tc.tile_pool(name="fin", bufs=3))

        io512 = cp.tile([P, G], F32)
        nc.sync.dma_start(out=io512[:], in_=iota512[:])
        r1sb = cp.tile([cfg.IN, 132], F32)
        nc.sync.dma_start(out=r1sb[:], in_=rhs1[:])
        r2sb = cp.tile([2 * HID, HID + 2], F32)
        nc.sync.dma_start(out=r2sb[:], in_=rhs2[:])
        fcsb = cp.tile([HID, 2], F32)
        nc.sync.dma_start(out=fcsb[:], in_=fcW[:])
        icsb = cp.tile([HID, G], F32)
        nc.sync.dma_start(out=icsb[:], in_=invc[:])
        gssb = cp.tile([P, CH], F32)
        nc.sync.dma_start(out=gssb[:], in_=gslot[:])
        idsb = cp.tile([P, P], F32)
        make_identity(nc, idsb[:])
        onesW = cp.tile([P, P], F32)
        nc.vector.memset(onesW[:], 1.0)
        out1 = o1p.tile([P, CH * P], F32)
        AD1 = cp.tile([P, CH * 2 * P], BF16)
        AD2 = cp.tile([P, CH * P], BF16)
        adcol1 = cp.tile([P, 2 * CH], F32)
        adcol2 = cp.tile([P, CH], F32)
        rows1 = cp.tile([P, CH * ROW1], BF16)  # local node rows (self-loop term)
        rows2 = cp.tile([P, CH * ROW2], BF16)
        nc.vector.memset(rows1[:], 0.0)
        nc.vector.memset(rows2[:], 0.0)

        # ---------------- dense 1: rows of table1 + ad columns ----------------
        for t in range(CH):
            xt = dio.tile([P, P], F32, tag="xt")
            nc.sync.dma_start(out=xt[:], in_=xT[:, t * P:(t + 1) * P])
            ps = dps.tile([P, 132], F32, tag="dtmp")
            nc.tensor.matmul(out=ps[:], lhsT=xt[:], rhs=r1sb[:], start=True, stop=True)
            upc = fin.tile([P, 2], F32, tag="upc")
            nc.scalar.activation(out=upc[:], in_=ps[:, 128:130], func=AF.Exp, scale=0.2)
            row = rows1[:, t * ROW1:(t + 1) * ROW1]
            nc.scalar.activation(out=row[:, 0:64], in_=ps[:, 0:64], func=AF.Copy,
                                 scale=upc[:, 0:1])
            nc.scalar.activation(out=row[:, 65:129], in_=ps[:, 64:128], func=AF.Copy,
                                 scale=upc[:, 1:2])
            nc.vector.tensor_copy(out=row[:, 64:65], in_=upc[:, 0:1])
            nc.vector.tensor_copy(out=row[:, 129:130], in_=upc[:, 1:2])
            um = row[:, UO1:UO1 + 12].bitcast(F32)
            nc.vector.tensor_copy(out=um[:, 0:2], in_=ps[:, 128:130])
            nc.vector.tensor_scalar(out=um[:, 2:4], in0=ps[:, 128:130],
                                    scalar1=0.8, scalar2=None, op0=ALU.mult)
            nc.vector.tensor_copy(out=um[:, 4:6], in_=upc[:])
            nc.vector.tensor_copy(out=adcol1[:, 2 * t:2 * t + 2], in_=ps[:, 130:132])
            nc.sync.dma_start(out=shard1[t * P:(t + 1) * P, :], in_=row[:])

        # AD: per (chunk, head) broadcast of ad over the free dim — replicate
        # the ad column along free (tensor_scalar vs ones), PE-transpose, copy.
        def build_ad(adcol_t, ncols, ad_t):
            for i in range(ncols):
                rc = fin.tile([P, P], F32, tag="rc")
                nc.vector.tensor_scalar(out=rc[:], in0=onesW[:],
                                        scalar1=adcol_t[:, i:i + 1], scalar2=None,
                                        op0=ALU.mult)
                vps = dps.tile([P, P], F32, tag="dtmp")
                nc.tensor.transpose(out=vps[:], in_=rc[:], identity=idsb[:])
                nc.scalar.copy(out=ad_t[:, i * P:(i + 1) * P], in_=vps[:])

        tc.strict_bb_all_engine_barrier()
        nc.gpsimd.collective_compute(
            "AllGather", ALU.bypass, replica_groups=[R],
            ins=[shard1[:]], outs=[table1[:]])
        build_ad(adcol1, 2 * CH, AD1)  # overlaps the AllGather

        # ---------------- edge phase (shared for both layers) ----------------
        def edge_layer(layer, tabA, tabB, ad_t, row_w, uo, nheads, finalize):
            for grp in groups:
                g0, ng = grp[0], len(grp)
                nlo, nhi = ng * SEC_LO, ng * SEC_HI
                glt = ixp.tile([P, nlo // 16], I16, tag="glt")
                nc.sync.dma_start(out=glt[:], in_=gl[:, g0 * SEC_LO // 16:(g0 * SEC_LO + nlo) // 16])
                ght = ixp.tile([P, nhi // 16], I16, tag="ght")
                nc.sync.dma_start(out=ght[:], in_=gh[:, g0 * SEC_HI // 16:(g0 * SEC_HI + nhi) // 16])
                s0t = s0pl.tile([P, ng * T * P], BF16, tag="s0t")
                nc.sync.dma_start(out=s0t[:], in_=s0p[:, g0 * T * P:(g0 + ng) * T * P])
                hgl = gp.tile([P, nlo // P, row_w], BF16, tag="hgl")
                paced_gather(xp, out_ap=hgl[:], in_ap=tabA, idxs_ap=glt[:],
                             num_idxs=nlo, num_idxs_reg=nlo, elem_size=row_w)
                hgh = gp.tile([P, nhi // P, row_w], BF16, tag="hgh")
                paced_gather(xp, out_ap=hgh[:], in_ap=tabB, idxs_ap=ght[:],
                             num_idxs=nhi, num_idxs_reg=nhi, elem_size=row_w)
                for ci, c in enumerate(grp):
                    Us = [ups.tile([P, 65], F32, tag=f"U{h}", name=f"U{h}")
                          for h in range(nheads)]
                    # tile PAIRS: acts are per tile (bias = per-edge as), but
                    # the max / mask ops and their fixed overheads run on
                    # [P, 256] spans (consecutive tiles' S0 are contiguous)
                    for tp_i in range(T // 2):
                        tpair = (2 * tp_i, 2 * tp_i + 1)
                        srcs = []
                        for t in tpair:
                            if t < T_LO:
                                hg_t, tt, nt = hgl, t, T_LO
                            else:
                                hg_t, tt, nt = hgh, t - T_LO, T_HI
                            srcs.append((hg_t, ci * nt + tt))
                        s0_t = s0t[:, ((ci * T) + tpair[0]) * P:((ci * T) + tpair[1] + 1) * P]
                        for h in range(nheads):
                            adsl = ad_t[:, (c * nheads + h) * P:(c * nheads + h + 1) * P]
                            Ep = sxp.tile([P, 2 * P], BF16, tag=f"E{h}")
                            for k, (hg_t, base) in enumerate(srcs):
                                uf = hg_t[:, base, uo:uo + 6 * nheads].bitcast(F32)
                                nc.scalar.activation(out=Ep[:, k * P:(k + 1) * P],
                                                     in_=adsl, func=AF.Exp, scale=0.8,
                                                     bias=uf[:, nheads + h:nheads + h + 1])
                            Gp = sxp.tile([P, 2 * P], BF16, tag=f"G{h}")
                            nc.vector.tensor_scalar(out=Gp[:], in0=Ep[:],
                                                    scalar1=1.0, scalar2=None,
                                                    op0=ALU.max)
                            Sh = sxp.tile([P, 2 * P], BF16, tag=f"Sh{h}")
                            nc.vector.tensor_tensor(out=Sh[:], in0=s0_t, in1=Gp[:],
                                                    op=ALU.mult)
                            for k, (hg_t, base) in enumerate(srcs):
                                t = tpair[k]
                                nc.tensor.matmul(
                                    out=Us[h][:], lhsT=Sh[:, k * P:(k + 1) * P],
                                    rhs=hg_t[:, base, h * 65:(h + 1) * 65],
                                    start=(t == 0), stop=(t == T - 1))
                    finalize(c, Us)

        # self-loop term: wt = e^{0.8 relu(as+ad)}, num += wt*(u'h), den += wt*u'
        def self_w(c, h, rows_t, adcol_t, row_w, uo, nheads):
            uf = rows_t[:, c * row_w + uo:c * row_w + uo + 6 * nheads].bitcast(F32)
            E = fin.tile([P, 1], F32, tag="Ew")
            nc.scalar.activation(out=E[:], in_=adcol_t[:, nheads * c + h:nheads * c + h + 1],
                                 func=AF.Exp, scale=0.8,
                                 bias=uf[:, nheads + h:nheads + h + 1])
            W = fin.tile([P, 1], F32, tag="Ww")
            nc.vector.tensor_scalar(out=W[:], in0=E[:], scalar1=1.0, scalar2=None,
                                    op0=ALU.max)
            wst = fin.tile([P, 1], F32, tag="wst")
            nc.vector.tensor_tensor(out=wst[:], in0=W[:],
                                    in1=uf[:, 2 * nheads + h:2 * nheads + h + 1],
                                    op=ALU.mult)
            numv = fin.tile([P, 64], F32, tag="numv")
            nc.vector.tensor_scalar(out=numv[:], in0=rows_t[:, c * row_w + h * 65:c * row_w + h * 65 + 64],
                                    scalar1=W[:], scalar2=None, op0=ALU.mult)
            return wst, numv

        def dense2_chunk(t):
            tp = dps.tile([P, P], F32, tag="dtmp")
            nc.tensor.transpose(out=tp[:], in_=out1[:, t * P:(t + 1) * P], identity=idsb[:])
            h1T = dio.tile([P, P], F32, tag="h1T")
            nc.scalar.copy(out=h1T[:], in_=tp[:])
            ps = dps.tile([P, HID + 2], F32, tag="dtmp")
            nc.tensor.matmul(out=ps[:], lhsT=h1T[:], rhs=r2sb[:], start=True, stop=True)
            upc = fin.tile([P, 1], F32, tag="upc2")
            nc.scalar.activation(out=upc[:], in_=ps[:, 64:65], func=AF.Exp, scale=0.2)
            row = rows2[:, t * ROW2:(t + 1) * ROW2]
            nc.scalar.activation(out=row[:, 0:64], in_=ps[:, 0:64], func=AF.Copy,
                                 scale=upc[:, 0:1])
            nc.vector.tensor_copy(out=row[:, 64:65], in_=upc[:, 0:1])
            um = row[:, UO2:UO2 + 6].bitcast(F32)
            nc.vector.tensor_copy(out=um[:, 0:1], in_=ps[:, 64:65])
            nc.vector.tensor_scalar(out=um[:, 1:2], in0=ps[:, 64:65],
                                    scalar1=0.8, scalar2=None, op0=ALU.mult)
            nc.vector.tensor_copy(out=um[:, 2:3], in_=upc[:])
            nc.vector.tensor_copy(out=adcol2[:, t:t + 1], in_=ps[:, 65:66])
            nc.sync.dma_start(out=shard2[t * P:(t + 1) * P, :], in_=row[:])

        def fin1(c, Us):
            for h in range(2):
                wst, numv = self_w(c, h, rows1, adcol1, ROW1, UO1, 2)
                un = fin.tile([P, 64], F32, tag="un1")
                nc.vector.tensor_tensor(out=un[:], in0=Us[h][:, 0:64], in1=numv[:],
                                        op=ALU.add)
                den = fin.tile([P, 1], F32, tag="den1")
                nc.vector.tensor_tensor(out=den[:], in0=Us[h][:, 64:65], in1=wst[:],
                                        op=ALU.add)
                rd = fin.tile([P, 1], F32, tag="rd1")
                nc.vector.reciprocal(out=rd[:], in_=den[:])
                nc.vector.tensor_scalar(
                    out=out1[:, c * P + h * 64:c * P + (h + 1) * 64],
                    in0=un[:], scalar1=rd[:], scalar2=0.0,
                    op0=ALU.mult, op1=ALU.max)
            dense2_chunk(c)  # layer-2 dense work rides along under edge-1

        edge_layer(1, table1[0:HALF, :], table1[HALF:NTAB, :], AD1, ROW1, UO1, 2, fin1)

        tc.strict_bb_all_engine_barrier()
        nc.gpsimd.collective_compute(
            "AllGather", ALU.bypass, replica_groups=[R],
            ins=[shard2[:]], outs=[table2[:]])
        build_ad(adcol2, CH, AD2)  # overlaps the AllGather

        # ---------------- edge layer 2 + pooling ----------------
        plT = pps.tile([HID, G], F32)

        def fin2(c, Us):
            wst, numv = self_w(c, 0, rows2, adcol2, ROW2, UO2, 1)
            un = fin.tile([P, 64], F32, tag="un2")
            nc.vector.tensor_tensor(out=un[:], in0=Us[0][:, 0:64], in1=numv[:],
                                    op=ALU.add)
            den = fin.tile([P, 1], F32, tag="den2")
            nc.vector.tensor_tensor(out=den[:], in0=Us[0][:, 64:65], in1=wst[:],
                                    op=ALU.add)
            rd = fin.tile([P, 1], F32, tag="rd2")
            nc.vector.reciprocal(out=rd[:], in_=den[:])
            o2 = fin.tile([P, HID], F32, tag="o2")
            nc.vector.tensor_scalar(out=o2[:], in0=un[:],
                                    scalar1=rd[:], scalar2=0.0,
                                    op0=ALU.mult, op1=ALU.max)
            sg = fin.tile([P, G], F32, tag="sg")
            nc.vector.tensor_scalar(out=sg[:], in0=io512[:],
                                    scalar1=gssb[:, c:c + 1], scalar2=None,
                                    op0=ALU.is_equal)
            nc.tensor.matmul(out=plT[:], lhsT=o2[:], rhs=sg[:],
                             start=(c == 0), stop=(c == CH - 1))

        edge_layer(2, table2[0:HALF, :], table2[HALF:NTAB, :], AD2, ROW2, UO2, 1, fin2)

        plsb = fin.tile([HID, G], F32)
        nc.vector.tensor_copy(out=plsb[:], in_=plT[:])
        nc.sync.dma_start(out=pool_loc[:], in_=plsb[:])
        tc.strict_bb_all_engine_barrier()
        nc.gpsimd.collective_compute(
            "AllReduce", ALU.add, replica_groups=[R],
            ins=[pool_loc[:]], outs=[pool_sh[:]])
        plr = fin.tile([HID, G], F32)
        nc.sync.dma_start(out=plr[:], in_=pool_sh[:])
        nc.vector.tensor_tensor(out=plr[:], in0=plr[:], in1=icsb[:], op=ALU.mult)
        for gt in range(max(1, G // P)):
            gw = min(P, G - gt * P)
            lg = dps.tile([P, 2], F32, tag="dtmp")
            nc.tensor.matmul(out=lg[:gw], lhsT=plr[:, gt * P:gt * P + gw], rhs=fcsb[:],
                             start=True, stop=True)
            mx = fin.tile([P, 1], F32, tag="mx")
            nc.vector.tensor_reduce(out=mx[:gw], in_=lg[:gw], op=ALU.max,
                                    axis=mybir.AxisListType.X)
            t1 = fin.tile([P, 2], F32, tag="t1")
            nc.vector.tensor_scalar(out=t1[:gw], in0=lg[:gw], scalar1=mx[:gw],
                                    scalar2=None, op0=ALU.subtract)
            ex = fin.tile([P, 2], F32, tag="ex")
            es = fin.tile([P, 1], F32, tag="es")
            nc.scalar.activation(out=ex[:gw], in_=t1[:gw], func=AF.Exp, accum_out=es[:gw])
            ln = fin.tile([P, 1], F32, tag="ln")
            nc.scalar.activation(out=ln[:gw], in_=es[:gw], func=AF.Ln)
            lsm = fin.tile([P, 2], F32, tag="lsm")
            nc.vector.tensor_scalar(out=lsm[:gw], in0=t1[:gw], scalar1=ln[:gw],
                                    scalar2=None, op0=ALU.subtract)
            nc.sync.dma_start(out=out_lg[gt * P:gt * P + gw, :], in_=lsm[:gw])

    nc.compile()
    return nc


# ------------------------------------------------------------------ entry

LAST_EXEC_NS = None

def kernel(x, edge_index, batch, W1, a_src1, a_dst1, b1, W2, a_src2, a_dst2, b2,
           fcW, fcb):
    x = np.asarray(x, np.float32)
    edge_index = np.asarray(edge_index, np.int64)
    batch = np.asarray(batch, np.int64)
    for b in (b1, b2, fcb):
        assert np.abs(np.asarray(b)).max() == 0.0, "nonzero bias unsupported"
    cfg = plan_cfg(N=x.shape[0], E0=edge_index.shape[1], G=512)
    in_maps = prep_inputs(cfg, x, edge_index, batch,
                          np.asarray(W1, np.float32), np.asarray(a_src1, np.float32),
                          np.asarray(a_dst1, np.float32), np.asarray(W2, np.float32),
                          np.asarray(a_src2, np.float32), np.asarray(a_dst2, np.float32),
                          np.asarray(fcW, np.float32))
    nc = build_nc(cfg)
    trace = os.environ.get("KERNEL_TRACE") == "1"
    res = run_bass_kernel_spmd(nc, in_maps, list(range(cfg.CORES)), trace=trace)
    global LAST_EXEC_NS
    LAST_EXEC_NS = res.exec_time_ns
    if trace:
        print(f"HW exec time: {res.exec_time_ns} ns "
              f"(mean {res.mean_exec_time_ns} ns)")
    return np.asarray(res.results[0]["out_lg"], np.float32)
